# revision 1
# baseline (speedup 1.0000x reference)
"""Trainium2 Bass kernel for ConvFCNet (3x conv+pool -> int8-fakequant FC + LIF SNN head).

Data-parallel over 8 NeuronCores: batch 512 -> 64 samples/core, weights replicated.

Per-core pipeline (all activations bf16, PSUM accumulation fp32, LIF state fp32):
  conv1 3->32  48x48, pad1 + relu + maxpool2 -> [32, 24, 24]
      im2col (27 = 3c x 9 taps) built by DMA into 4 groups at 27-row pitch plus a
      shared const-1 bias row: ONE block-diagonal matmul (K=109, M=128) per 8-row band.
  conv2 32->64 24x24 -> [64, 12, 12]
      im2col over dx (96 = 32c x 3dx) + const-1 bias row (K=97); dy folded into
      matmul free-dim offsets; 3 accumulating matmuls, 2 sample-halves via col tiling.
  conv3 64->128 12x12 -> [128, 6, 6]
      im2col over dx: A=[128 = 64c x dx01], B=[65 = c,dx2 + bias row]; 6 accumulating MMs.
  maxpool: most tiles Act-evacuate (relu+copy PSUM->SBUF bf16, x-deinterleaved)
      then two 4x-rate DVE max stages writing straight into the next layer's padded
      buffer; the rest use direct DVE tensor_reduce from PSUM with deferred relu
      (balances Act vs DVE; Pool/GPSIMD cannot touch PSUM or run DVE opcodes on HW).
  FC1 4608->512 weight-stationary: out [unit, sample] built directly (no transposes),
      4 unit-groups x 36 k-chunks of N=64.
  LIF dynamics (tau=2, vth=1, hard reset) + FC2 512->128 + FC3 128->5, 3 timesteps,
      all in [unit, sample] orientation.
"""

import numpy as np
import ml_dtypes

import concourse.bass as bass
import concourse.bacc as bacc
import concourse.tile as tile
import concourse.mybir as mybir

AF = mybir.ActivationFunctionType
ALU = mybir.AluOpType
BF16 = mybir.dt.bfloat16
F32 = mybir.dt.float32

NCORES = 8
B = 64  # samples per core


def _v(ap, p0, npart, dims, off=0):
    """View into an SBUF/PSUM tile AP: partition slice [p0, p0+npart) + custom free dims."""
    pitch = ap.ap[0][0]
    return bass.AP(
        tensor=ap.tensor,
        offset=ap.offset + p0 * pitch + off,
        ap=[[pitch, npart]] + [list(d) for d in dims],
    )


def _dv(ap, off, dims):
    """View into a DRAM tensor AP with custom dims."""
    return bass.AP(tensor=ap.tensor, offset=ap.offset + off, ap=[list(d) for d in dims])


def _emit(tc, io):
    nc = tc.nc
    from contextlib import ExitStack

    with ExitStack() as ctx:
        # ---------------- persistent buffers + weights ----------------
        wp = ctx.enter_context(tc.tile_pool(name="wts", bufs=1))
        w1sb = wp.tile([109, 128], BF16)
        nc.gpsimd.dma_start(w1sb[:, :], io["w1l"][:, :])
        w2sb = wp.tile([97, 192], BF16)
        nc.gpsimd.dma_start(w2sb[:, :], io["w2l"][:, :])
        w3asb = wp.tile([128, 384], BF16)
        nc.gpsimd.dma_start(w3asb[:, :], io["w3a"][:, :])
        w3bsb = wp.tile([65, 384], BF16)
        nc.gpsimd.dma_start(w3bsb[:, :], io["w3b"][:, :])
        wf2sb = wp.tile([128, 512], BF16)
        nc.gpsimd.dma_start(wf2sb[:, :], io["wf2"][:, :])
        wf3sb = wp.tile([128, 5], BF16)
        nc.gpsimd.dma_start(wf3sb[:, :], io["wf3"][:, :])

        mp = ctx.enter_context(tc.tile_pool(name="main", bufs=1))
        # conv2 pooled output, padded 14x14, partition 64h+c holds samples of parity h
        xpad3 = mp.tile([128, 32 * 198 + 4], BF16)
        for dims, off in [
            ([[198, 32], [1, 14]], 0),        # top row
            ([[198, 32], [1, 14]], 182),      # bottom row
            ([[198, 32], [14, 14]], 0),       # left col
            ([[198, 32], [14, 14]], 13),      # right col
            ([[1, 4]], 32 * 198),             # tail pad (im2col dx over-read)
            ([[198, 32], [1, 2]], 196),       # per-sample slack (pitch 198 vs 196)
        ]:
            nc.gpsimd.memset(_v(xpad3, 0, 128, dims, off), 0.0)
        # conv3 pooled output (features): [128c, sample*36 + hw]
        feat = mp.tile([128, B * 36], BF16)

        # Scoped pools: xpad2 frees SBUF after conv1 (its last readers are the
        # buf96 DMAs emitted inside the conv1 loop); buf96 stays until the end.
        b96pool = ctx.enter_context(tc.tile_pool(name="b96", bufs=1))
        xp2cm = tc.tile_pool(name="xp2", bufs=1)
        xp2pool = xp2cm.__enter__()
        # conv1 pooled output, padded 26x26, partition 32q+c holds samples 16q..16q+15
        xpad2 = xp2pool.tile([128, 16 * 676 + 4], BF16)
        for dims, off in [
            ([[676, 16], [1, 26]], 0),        # top row
            ([[676, 16], [1, 26]], 650),      # bottom row
            ([[676, 16], [26, 26]], 0),       # left col
            ([[676, 16], [26, 26]], 25),      # right col
            ([[1, 4]], 16 * 676),             # tail pad (im2col dx over-read)
        ]:
            nc.gpsimd.memset(_v(xpad2, 0, 128, dims, off), 0.0)
        # conv2 im2col buffer (96 rows = 32c x 3dx, row 96 = const 1 bias row)
        buf96 = b96pool.tile([97, B * 676 + 4], BF16)
        nc.gpsimd.dma_start(_v(buf96, 96, 1, [[1, B * 676 + 4]]), io["ones"][0:1, 0 : B * 676 + 4])

        # ---------------- conv1 + conv2 (software-pipelined emission) ----------------
        # Engine queues are in-order, so conv2 matmuls are emitted interleaved
        # between conv1 samples (one conv1 half behind) to keep PE fed while
        # conv1's pooling drains. conv2 block b needs buf96 samples 2b..2b+1,
        # i.e. conv1 half b//8, whose buf96 chunk DMA is emitted with that half.
        with (
            tc.tile_pool(name="c1imc", bufs=2) as c1i,
            tc.tile_pool(name="c1ps", bufs=4, space="PSUM") as c1p,
            tc.tile_pool(name="c1t", bufs=4) as c1t,
        ):
            def conv1_sample(imc, half, s):
                hs = half * 4 + s
                for yt in range(6):
                    ps = c1p.tile([128, 384], F32, tag="ps1", name="ps1")
                    # one block-diagonal matmul: K=109 (4x27 taps + bias row), M=128
                    nc.tensor.matmul(
                        ps[:, :],
                        _v(w1sb, 0, 109, [[1, 128]]),
                        _v(imc, 0, 109, [[50, 8], [1, 48]], s * 2500 + yt * 400),
                        start=True,
                        stop=True,
                    )
                    xp_dst = _v(xpad2, 0, 128, [[26, 4], [1, 24]], hs * 676 + 27 + yt * 104)
                    if yt < 4:
                        # Act-evacuate: relu+copy, x-deinterleaved (phase, y, xh)
                        stg = c1t.tile([128, 384], BF16, tag="stg", name="stg")
                        nc.scalar.activation(
                            _v(stg, 0, 128, [[24, 8], [1, 24], [192, 2]]),
                            ps[:, :], AF.Relu,
                        )
                        # max stages on DVE at 4x (Pool lacks these opcodes on HW)
                        xm = c1t.tile([128, 192], BF16, tag="xm", name="xm")
                        nc.vector.scalar_tensor_tensor(
                            _v(xm, 0, 128, [[1, 192]]),
                            _v(stg, 0, 128, [[1, 192]]), 0.0,
                            _v(stg, 0, 128, [[1, 192]], 192),
                            ALU.max, ALU.max,
                        )
                        nc.vector.scalar_tensor_tensor(
                            xp_dst,
                            _v(xm, 0, 128, [[48, 4], [1, 24]]), 0.0,
                            _v(xm, 0, 128, [[48, 4], [1, 24]], 24),
                            ALU.max, ALU.max,
                        )
                    else:
                        # DVE: direct 2x2 max-reduce from PSUM (relu deferred)
                        nc.vector.tensor_reduce(
                            xp_dst,
                            _v(ps, 0, 128, [[96, 4], [2, 24], [48, 2], [1, 2]]),
                            mybir.AxisListType.XY,
                            ALU.max,
                        )
                # in-place relu over the DVE-path rows (16..23) of this sample
                rows = _v(xpad2, 0, 128, [[26, 8], [1, 24]], hs * 676 + 27 + 16 * 26)
                nc.vector.tensor_scalar(rows, rows, 0.0, None, ALU.max)

            for half in range(4):
                imc = c1i.tile([109, 4 * 2500 + 4], BF16, tag="imc", name="imc")
                for q in range(4):
                    for dy in range(3):
                        src = _dv(
                            io["xpad"],
                            (q * 16 + half * 4) * 2500 + dy * 50,
                            [[1, 3], [160128, 3], [1, 10000]],
                        )
                        dst = _v(imc, 27 * q + dy * 9, 9, [[1, 10000]])
                        nc.sync.dma_start(dst, src)
                nc.sync.dma_start(_v(imc, 108, 1, [[1, 10000]]), io["ones"][0:1, 0:10000])
                for s in range(4):
                    conv1_sample(imc, half, s)
                # buf96 chunk for this half (qt = half), on the Pool DMA queue
                for j in range(4):
                    src = _v(xpad2, 32 * j, 32, [[1, 3], [1, 4 * 676]], half * 4 * 676)
                    dst = _v(buf96, 0, 96, [[1, 4 * 676]], (j * 16 + half * 4) * 676)
                    nc.gpsimd.dma_start(dst, src)
        xp2cm.__exit__(None, None, None)

        # FC1 weights: 4.7MB load lands during conv2 / conv3
        fcw = ctx.enter_context(tc.tile_pool(name="fcw", bufs=1))
        wf1sb = fcw.tile([128, 18432], BF16)
        nc.gpsimd.dma_start(wf1sb[:, :], io["wf1"][:, :])

        # conv3 im2col buffers (double-buffered per parity h; row 64 of B = bias row)
        c3b = ctx.enter_context(tc.tile_pool(name="c3buf", bufs=1))
        bufA = [c3b.tile([128, 32 * 198 + 4], BF16, name=f"bufA{h}") for h in range(2)]
        bufB = [c3b.tile([65, 32 * 198 + 4], BF16, name=f"bufB{h}") for h in range(2)]
        for h in range(2):
            nc.sync.dma_start(_v(bufB[h], 64, 1, [[1, 32 * 198 + 4]]), io["ones"][0:1, 0 : 32 * 198 + 4])

        # ---------------- conv2 ----------------
        with (
            tc.tile_pool(name="c2ps", bufs=4, space="PSUM") as c2p,
            tc.tile_pool(name="c2t", bufs=4) as c2t,
        ):
            def conv2_b(b):
                bi = b % 4
                for yh in range(2):
                    ps = c2p.tile([128, 288], F32, tag="ps2", name="ps2")
                    for h in range(2):
                        for dy in range(3):
                            rhs = _v(
                                buf96, 0, 97, [[26, 12], [1, 24]],
                                (2 * b + h) * 676 + yh * 312 + dy * 26,
                            )
                            nc.tensor.matmul(
                                ps[64 * h : 64 * h + 64, :],
                                w2sb[0:97, dy * 64 : dy * 64 + 64],
                                rhs,
                                start=(dy == 0),
                                stop=(dy == 2),
                                tile_position=(0, 64 * h),
                            )
                    xp_dst = _v(xpad3, 0, 128, [[14, 6], [1, 12]], b * 198 + 15 + yh * 84)
                    if bi < 3:
                        # Act-evacuate: relu+copy x-deinterleaved (phase, y, xh)
                        stg = c2t.tile([128, 288], BF16, tag="stg", name="stg")
                        nc.scalar.activation(
                            _v(stg, 0, 128, [[12, 12], [1, 12], [144, 2]]),
                            ps[:, :], AF.Relu,
                        )
                        xm = c2t.tile([128, 144], BF16, tag="xm", name="xm")
                        nc.vector.scalar_tensor_tensor(
                            _v(xm, 0, 128, [[1, 144]]),
                            _v(stg, 0, 128, [[1, 144]]), 0.0,
                            _v(stg, 0, 128, [[1, 144]], 144),
                            ALU.max, ALU.max,
                        )
                        nc.vector.scalar_tensor_tensor(
                            xp_dst,
                            _v(xm, 0, 128, [[24, 6], [1, 12]]), 0.0,
                            _v(xm, 0, 128, [[24, 6], [1, 12]], 12),
                            ALU.max, ALU.max,
                        )
                    else:
                        # DVE direct reduce path (relu applied in-place after yh=1)
                        nc.vector.tensor_reduce(
                            xp_dst,
                            _v(ps, 0, 128, [[48, 6], [2, 12], [24, 2], [1, 2]]),
                            mybir.AxisListType.XY,
                            ALU.max,
                        )
                if bi == 3:
                    rows = _v(xpad3, 0, 128, [[14, 12], [1, 12]], b * 198 + 15)
                    nc.vector.tensor_scalar(rows, rows, 0.0, None, ALU.max)
                # conv3 im2col chunk once its xpad3 sample range is complete (SP queue)
                if b == 15 or b == 31:
                    ck = b // 16
                    off = ck * 16 * 198
                    for h in range(2):
                        nc.sync.dma_start(
                            _v(bufA[h], 0, 128, [[1, 16 * 198]], off),
                            _v(xpad3, 64 * h, 64, [[1, 2], [1, 16 * 198]], off),
                        )
                        nc.sync.dma_start(
                            _v(bufB[h], 0, 64, [[1, 16 * 198]], off),
                            _v(xpad3, 64 * h, 64, [[1, 16 * 198]], off + 2),
                        )

            for b in range(32):
                conv2_b(b)

        # ---------------- conv3 + FC1 (parity-pipelined) ----------------
        cur1p = ctx.enter_context(tc.tile_pool(name="cur1p", bufs=1, space="PSUM"))
        cur1 = cur1p.tile([128, 256], F32)
        with (
            tc.tile_pool(name="c3ps", bufs=4, space="PSUM") as c3p,
            tc.tile_pool(name="c3t", bufs=4) as c3t,
        ):
            for h in range(2):
                for bq in range(4):
                    for bj in range(4):
                        bp = bq * 4 + bj
                        ps = c3p.tile([128, 288], F32, tag="ps3")
                        for dy in range(3):
                            dims = [[198, 2], [14, 12], [1, 12]]
                            off = bp * 2 * 198 + dy * 14
                            nc.tensor.matmul(
                                ps[:, :], w3asb[0:128, dy * 128 : dy * 128 + 128],
                                _v(bufA[h], 0, 128, dims, off),
                                start=(dy == 0), stop=False,
                            )
                            nc.tensor.matmul(
                                ps[:, :], w3bsb[0:65, dy * 128 : dy * 128 + 128],
                                _v(bufB[h], 0, 65, dims, off),
                                start=False, stop=(dy == 2),
                            )
                        # slot of (h, bp, b) is sample 4bp+h+2b -> feat col (4bp+h+2b)*36
                        if bj < 3:
                            # Act-evacuate: relu+copy x-deinterleaved (phase, b, y, xh)
                            stg = c3t.tile([128, 288], BF16, tag="stg")
                            nc.scalar.activation(
                                _v(stg, 0, 128, [[72, 2], [6, 12], [1, 6], [144, 2]]),
                                ps[:, :], AF.Relu,
                            )
                            xm = c3t.tile([128, 144], BF16, tag="xm")
                            nc.vector.scalar_tensor_tensor(
                                _v(xm, 0, 128, [[1, 144]]),
                                _v(stg, 0, 128, [[1, 144]]), 0.0,
                                _v(stg, 0, 128, [[1, 144]], 144),
                                ALU.max, ALU.max,
                            )
                            nc.vector.scalar_tensor_tensor(
                                _v(feat, 0, 128, [[72, 2], [6, 6], [1, 6]], (4 * bp + h) * 36),
                                _v(xm, 0, 128, [[72, 2], [12, 6], [1, 6]]), 0.0,
                                _v(xm, 0, 128, [[72, 2], [12, 6], [1, 6]], 6),
                                ALU.max, ALU.max,
                            )
                        else:
                            # DVE direct reduce per sample (relu deferred to feat pass)
                            for i in range(2):
                                nc.vector.tensor_reduce(
                                    _v(feat, 0, 128, [[6, 6], [1, 6]], (4 * bp + h + 2 * i) * 36),
                                    _v(ps, 0, 128, [[24, 6], [2, 6], [12, 2], [1, 2]], i * 144),
                                    mybir.AxisListType.XY,
                                    ALU.max,
                                )
                # in-place relu over the DVE-reduced feat slots of this parity
                # (bp = 3,7,11,15 -> slots 4bp+h and 4bp+2+h)
                rows = _v(feat, 0, 128, [[576, 4], [72, 2], [1, 36]], (12 + h) * 36)
                nc.vector.tensor_scalar(rows, rows, 0.0, None, ALU.max)
                # FC1 for parity h: out [unit, 32 samples] at cur1 col 64g+32h
                # (samples of parity h = feat cols h, h+2, ... -> stride 72)
                for g in range(4):
                    for k in range(36):
                        nc.tensor.matmul(
                            cur1[:, 64 * g + 32 * h : 64 * g + 32 * h + 32],
                            wf1sb[:, k * 512 + g * 128 : k * 512 + g * 128 + 128],
                            _v(feat, 0, 128, [[72, 32]], k + 36 * h),
                            start=(k == 0),
                            stop=(k == 35),
                        )

        # ---------------- LIF + FC2/FC3 ----------------
        with (
            tc.tile_pool(name="cur2p", bufs=2, space="PSUM") as cur2p,
            tc.tile_pool(name="lif", bufs=1) as lifp,
            tc.tile_pool(name="liftmp", bufs=2) as dtp,
        ):
            cur1sb = lifp.tile([128, 256], BF16)
            nc.scalar.activation(cur1sb[:, :], cur1[:, :], AF.Copy)

            zeros256 = lifp.tile([128, 256], F32)
            nc.gpsimd.memset(zeros256[:, :], 0.0)
            v1 = lifp.tile([128, 256], F32)
            s1 = lifp.tile([128, 256], BF16)
            nc.gpsimd.memset(v1[:, :], 0.0)
            v2 = lifp.tile([128, 64], F32)
            nc.gpsimd.memset(v2[:, :], 0.0)
            s2 = lifp.tile([128, 64], BF16)
            v3 = lifp.tile([5, 64], F32)
            nc.gpsimd.memset(v3[:, :], 0.0)
            acc = lifp.tile([5, 64], F32)
            nc.gpsimd.memset(acc[:, :], 0.0)

            def lif_step(v, cur, s_out):
                # v <- v + (cur - v)*0.5 ; s = (v >= 1) ; v <- 0 where s
                n = v.shape[1]
                d = dtp.tile([v.shape[0], n], F32, tag="d", name="d")
                nc.vector.tensor_tensor(d[:, :], cur[:, :], v[:, :], ALU.subtract)
                nc.vector.scalar_tensor_tensor(v[:, :], d[:, :], 0.5, v[:, :], ALU.mult, ALU.add)
                nc.vector.tensor_scalar(s_out[:, :], v[:, :], 1.0, None, ALU.is_ge)
                mask = s_out[:, :].bitcast(mybir.dt.uint16 if s_out.dtype == BF16 else mybir.dt.uint32)
                nc.vector.copy_predicated(v[:, :], mask, zeros256[0 : v.shape[0], 0 : n])

            for t in range(3):
                lif_step(v1, cur1sb, s1)
                cur2 = cur2p.tile([128, 64], F32, tag="cur2")
                for g in range(4):
                    nc.tensor.matmul(
                        cur2[:, :], wf2sb[:, g * 128 : g * 128 + 128], s1[:, 64 * g : 64 * g + 64],
                        start=(g == 0), stop=(g == 3),
                    )
                lif_step(v2, cur2, s2)
                cur3 = cur2p.tile([5, 64], F32, tag="cur3")
                nc.tensor.matmul(cur3[0:5, :], wf3sb[0:128, 0:5], s2[:, :], start=True, stop=True)
                s3 = dtp.tile([5, 64], F32, tag="s3")
                d3 = dtp.tile([5, 64], F32, tag="d3")
                nc.vector.tensor_tensor(d3[:, :], cur3[0:5, :], v3[:, :], ALU.subtract)
                nc.vector.scalar_tensor_tensor(v3[:, :], d3[:, :], 0.5, v3[:, :], ALU.mult, ALU.add)
                nc.vector.tensor_scalar(s3[:, :], v3[:, :], 1.0, None, ALU.is_ge)
                nc.vector.copy_predicated(v3[:, :], s3[:, :].bitcast(mybir.dt.uint32), zeros256[0:5, 0:64])
                nc.vector.tensor_tensor(acc[:, :], acc[:, :], s3[:, :], ALU.add)

            # acc/3 for acc in {0,1,2,3}: mult by fp32(1/3) matches true division except acc=3
            # (3*0.33333334 = 1.0000001) -> clamp with min(., 1.0) for exactness.
            nc.vector.tensor_scalar(acc[:, :], acc[:, :], float(np.float32(1.0) / np.float32(3.0)), 1.0, ALU.mult, ALU.min)
            # acc col (32h + j) holds sample 2j+h -> un-permute on the way out
            for h in range(2):
                nc.sync.dma_start(
                    _dv(io["out"], h, [[64, 5], [2, 32]]),
                    _v(acc, 0, 5, [[1, 32]], 32 * h),
                )


def _build():
    nc = bacc.Bacc("TRN2", target_bir_lowering=False, debug=False, enable_asserts=True)
    io = {}

    def inp(name, shape, dt):
        io[name] = nc.dram_tensor(name, shape, dt, kind="ExternalInput").ap()

    inp("xpad", [3, 160128], BF16)
    inp("w1l", [109, 128], BF16)
    inp("w2l", [97, 192], BF16)
    inp("w3a", [128, 384], BF16)
    inp("w3b", [65, 384], BF16)
    inp("wf1", [128, 18432], BF16)
    inp("wf2", [128, 512], BF16)
    inp("wf3", [128, 5], BF16)
    inp("ones", [1, 43268], BF16)
    io["out"] = nc.dram_tensor("out", [5, 64], F32, kind="ExternalOutput").ap()

    import os
    unroll = int(os.environ.get("KERNEL_UNROLL", "1"))
    with tile.TileContext(nc) as tc:
        for _ in range(unroll):
            _emit(tc, io)
    nc.compile()
    return nc


def _fake_quant(w):
    w = np.asarray(w, np.float32)
    scale = np.float32(np.max(np.abs(w)) / np.float32(127.0))
    wq = np.clip(np.round(w / scale), -127.0, 127.0).astype(np.float32) * scale
    return wq.astype(np.float32)


def _bf16(a):
    return np.asarray(a, np.float32).astype(ml_dtypes.bfloat16)


def _prep_weights(conv1_w, conv1_b, conv2_w, conv2_b, conv3_w, conv3_b, W1, W2, W3):
    c1 = np.asarray(conv1_w, np.float32)  # [32, 3, 3, 3]
    c2 = np.asarray(conv2_w, np.float32)  # [64, 32, 3, 3]
    c3 = np.asarray(conv3_w, np.float32)  # [128, 64, 3, 3]

    # conv1 block-diagonal: rows 27q..27q+26 = taps of group q -> cols 32q..32q+31;
    # row 108 = bias (tiled 4x over the 4 col groups).
    w1l = np.zeros((109, 128), np.float32)
    wk = c1.transpose(2, 3, 1, 0).reshape(27, 32)  # [(dy,dx,c), m]
    for q in range(4):
        w1l[27 * q : 27 * q + 27, 32 * q : 32 * q + 32] = wk
    w1l[108, :] = np.tile(np.asarray(conv1_b, np.float32), 4)

    w2l = np.zeros((97, 192), np.float32)
    w2l[0:96] = c2.transpose(1, 3, 2, 0).reshape(96, 192)  # [(c,dx), (dy,m)]
    w2l[96, 0:64] = np.asarray(conv2_b, np.float32)        # bias rides the dy=0 block

    w3x = c3.transpose(1, 3, 2, 0)  # [c, dx, dy, m]
    w3a = w3x[:, 0:2].reshape(128, 384)
    w3b = np.zeros((65, 384), np.float32)
    w3b[0:64] = w3x[:, 2].reshape(64, 384)
    w3b[64, 0:128] = np.asarray(conv3_b, np.float32)       # bias rides the dy=0 block

    W1q = _fake_quant(W1)  # [512, 4608]
    W2q = _fake_quant(W2)  # [128, 512]
    W3q = _fake_quant(W3)  # [5, 128]

    # [c, k*512 + u] = W1q[u, c*36 + k]  (FC1 weight-stationary: out [unit, sample])
    wf1 = W1q.reshape(512, 128, 36).transpose(1, 2, 0).reshape(128, 36 * 512)
    wf2 = W2q.T.reshape(4, 128, 128).transpose(1, 0, 2).reshape(128, 512)
    wf3 = W3q.T.copy()  # [128, 5]

    return {
        "w1l": _bf16(w1l),
        "w2l": _bf16(w2l),
        "w3a": _bf16(w3a),
        "w3b": _bf16(w3b),
        "wf1": _bf16(wf1),
        "wf2": _bf16(wf2),
        "wf3": _bf16(wf3),
        "ones": _bf16(np.ones((1, 43268), np.float32)),
    }


_NC = None
LAST_RESULTS = None


def kernel(x, conv1_w, conv1_b, conv2_w, conv2_b, conv3_w, conv3_b, W1, W2, W3, _trace=False):
    global _NC, LAST_RESULTS
    if _NC is None:
        _NC = _build()

    wmap = _prep_weights(conv1_w, conv1_b, conv2_w, conv2_b, conv3_w, conv3_b, W1, W2, W3)

    x = np.asarray(x, np.float32)
    xp = np.zeros((512, 3, 50, 50), np.float32)
    xp[:, :, 1:49, 1:49] = x
    in_maps = []
    for i in range(NCORES):
        shard = xp[B * i : B * (i + 1)].transpose(1, 0, 2, 3).reshape(3, B * 2500)
        sp = np.zeros((3, 160128), np.float32)
        sp[:, : B * 2500] = shard
        in_maps.append({"xpad": _bf16(sp), **wmap})

    from concourse.bass_utils import run_bass_kernel_spmd

    res = run_bass_kernel_spmd(_NC, in_maps, core_ids=list(range(NCORES)), trace=_trace)
    LAST_RESULTS = res
    out = np.concatenate([np.asarray(res.results[i]["out"]).T for i in range(NCORES)], axis=0)
    return np.ascontiguousarray(out.astype(np.float32))



# revision 3
# speedup vs baseline: 1.1448x; 1.1448x over previous
"""Trainium2 Bass kernel for ConvFCNet (3x conv+pool -> int8-fakequant FC + LIF SNN head).

Data-parallel over 8 NeuronCores: batch 512 -> 64 samples/core, weights replicated.

v1 rework (from 156us baseline): the PE queue is kept continuously fed so the
tensor engine stays at full p-state and is the binding resource (~89us of
matmul work):
  - conv1 im2col is built on the HOST in per-chunk order (chunk m = samples
    4m..4m+3 via the block-diagonal group trick), DMAed in 16 fine-grained
    chunks so the first matmul starts at ~4us instead of 11us.
  - conv2 blocks are emitted interleaved into the conv1 chunk loop (lag 2),
    so conv2 matmuls run while conv1 pooling drains instead of after it.
  - pooling max stages use tensor_tensor(max) (2x DVE perf mode for packed
    bf16) instead of scalar_tensor_tensor (no perf mode), and PSUM tiles span
    2 banks so one Act evacuation covers 2 matmul tiles.
  - LIF layer-1 is solved analytically across the 3 timesteps straight from
    the cur1 PSUM (s1_t thresholds 2, 4/3, 8/7 on cur1), FC2 runs all 3
    timesteps in one matmul set (N=192), and FC2/FC3 weights are pre-scaled
    by 0.5 on the host so the LIF v-update is a single scalar_tensor_tensor.
"""

import numpy as np
import ml_dtypes

import concourse.bass as bass
import concourse.bacc as bacc
import concourse.tile as tile
import concourse.mybir as mybir

AF = mybir.ActivationFunctionType
ALU = mybir.AluOpType
BF16 = mybir.dt.bfloat16
F32 = mybir.dt.float32

NCORES = 8
B = 64  # samples per core


def _v(ap, p0, npart, dims, off=0):
    """View into an SBUF/PSUM tile AP: partition slice [p0, p0+npart) + custom free dims."""
    pitch = ap.ap[0][0]
    return bass.AP(
        tensor=ap.tensor,
        offset=ap.offset + p0 * pitch + off,
        ap=[[pitch, npart]] + [list(d) for d in dims],
    )


def _dv(ap, off, dims):
    """View into a DRAM tensor AP with custom dims."""
    return bass.AP(tensor=ap.tensor, offset=ap.offset + off, ap=[list(d) for d in dims])


def _emit(tc, io):
    nc = tc.nc
    from contextlib import ExitStack

    with ExitStack() as ctx:
        # ---------------- persistent buffers + weights ----------------
        wp = ctx.enter_context(tc.tile_pool(name="wts", bufs=1))
        w1sb = wp.tile([109, 128], BF16)
        nc.gpsimd.dma_start(w1sb[:, :], io["w1l"][:, :])
        w2sb = wp.tile([97, 192], BF16)
        nc.gpsimd.dma_start(w2sb[:, :], io["w2l"][:, :])
        w3asb = wp.tile([128, 384], BF16)
        nc.gpsimd.dma_start(w3asb[:, :], io["w3a"][:, :])
        w3bsb = wp.tile([65, 384], BF16)
        nc.gpsimd.dma_start(w3bsb[:, :], io["w3b"][:, :])
        wf2sb = wp.tile([128, 512], BF16)
        nc.gpsimd.dma_start(wf2sb[:, :], io["wf2"][:, :])
        wf3sb = wp.tile([128, 5], BF16)
        nc.gpsimd.dma_start(wf3sb[:, :], io["wf3"][:, :])

        # preload the Relu activation table while the head DMAs run
        scr = wp.tile([1, 8], BF16)
        nc.scalar.activation(_v(scr, 0, 1, [[1, 8]]), _v(w1sb, 0, 1, [[1, 8]]), AF.Relu)

        mp = ctx.enter_context(tc.tile_pool(name="main", bufs=1))
        # conv1 pooled output, padded 26x26; partition 32g+c = sample 4m+g at col m*676
        xpad2 = mp.tile([128, 16 * 676 + 4], BF16)
        for dims, off in [
            ([[676, 16], [1, 26]], 0),        # top row
            ([[676, 16], [1, 26]], 650),      # bottom row
            ([[676, 16], [26, 26]], 0),       # left col
            ([[676, 16], [26, 26]], 25),      # right col
            ([[1, 4]], 16 * 676),             # tail pad (im2col dx over-read)
        ]:
            nc.gpsimd.memset(_v(xpad2, 0, 128, dims, off), 0.0)
        # conv2 pooled output, padded 14x14, partition 64h+c holds samples of parity h
        xpad3 = mp.tile([128, 32 * 198 + 4], BF16)
        for dims, off in [
            ([[198, 32], [1, 14]], 0),        # top row
            ([[198, 32], [1, 14]], 182),      # bottom row
            ([[198, 32], [14, 14]], 0),       # left col
            ([[198, 32], [14, 14]], 13),      # right col
            ([[1, 4]], 32 * 198),             # tail pad (im2col dx over-read)
            ([[198, 32], [1, 2]], 196),       # per-sample slack (pitch 198 vs 196)
        ]:
            nc.gpsimd.memset(_v(xpad3, 0, 128, dims, off), 0.0)
        # conv3 pooled output (features): [128c, sample*36 + hw]
        feat = mp.tile([128, B * 36], BF16)

        # LIF state (hoisted memsets run during the DMA head)
        lifp = ctx.enter_context(tc.tile_pool(name="lif", bufs=1))
        zeros = lifp.tile([128, 64], F32)
        nc.gpsimd.memset(zeros[:, :], 0.0)
        v2 = lifp.tile([128, 64], F32)
        nc.gpsimd.memset(v2[:, :], 0.0)
        v3 = lifp.tile([5, 64], F32)
        nc.gpsimd.memset(v3[:, :], 0.0)
        acc = lifp.tile([5, 64], F32)
        nc.gpsimd.memset(acc[:, :], 0.0)
        s1_all = lifp.tile([128, 768], BF16)   # [t*256 + cur1-col]
        s2_all = lifp.tile([128, 192], BF16)   # [t*64 + sample-col]

        # conv3 im2col buffers (row 64 of B = bias row)
        c3b = ctx.enter_context(tc.tile_pool(name="c3buf", bufs=1))
        bufA = [c3b.tile([128, 32 * 198 + 4], BF16, name=f"bufA{h}") for h in range(2)]
        bufB = [c3b.tile([65, 32 * 198 + 4], BF16, name=f"bufB{h}") for h in range(2)]
        for h in range(2):
            nc.sync.dma_start(_v(bufB[h], 64, 1, [[1, 32 * 198 + 4]]), io["ones"][0:1, 0 : 32 * 198 + 4])

        # conv2 im2col quarters (96 rows = 32c x 3dx, row 96 = bias row), scoped
        b96 = ctx.enter_context(tc.tile_pool(name="b96", bufs=2))
        bqs = {}

        # ---------------- conv1 + conv2 (interleaved, PE stays fed) ----------------
        with (
            tc.tile_pool(name="c1imc", bufs=2) as c1i,
            tc.tile_pool(name="c1ps", bufs=2, space="PSUM") as c1p,
            tc.tile_pool(name="c1t", bufs=3) as c1t,
            tc.tile_pool(name="c2ps", bufs=2, space="PSUM") as c2p,
            tc.tile_pool(name="c2t", bufs=3) as c2t,
        ):
            def conv1_chunk(m):
                imct = c1i.tile([109, 2500], BF16, tag="imc", name="imc")
                nc.sync.dma_start(
                    _v(imct, 0, 109, [[1, 2500]]),
                    _dv(io["imc"], m * 2500, [[40000, 109], [1, 2500]]),
                )
                base = m * 676 + 27
                for tj in range(3):
                    ps = c1p.tile([128, 1024], F32, tag="ps1", name="ps1")
                    for u in range(2):
                        nc.tensor.matmul(
                            _v(ps, 0, 128, [[1, 384]], u * 512),
                            _v(w1sb, 0, 109, [[1, 128]]),
                            _v(imct, 0, 109, [[50, 8], [1, 48]], (tj * 2 + u) * 400),
                            start=True,
                            stop=True,
                        )
                    if tj < 2:
                        # Act evac: relu+copy both banks, x-deinterleaved (u,y,xh,phase)
                        stg = c1t.tile([128, 768], BF16, tag="stg", name="stg")
                        nc.scalar.activation(
                            _v(stg, 0, 128, [[192, 2], [24, 8], [1, 24], [384, 2]]),
                            _v(ps, 0, 128, [[512, 2], [48, 8], [2, 24], [1, 2]]),
                            AF.Relu,
                        )
                        # max stages as tensor_tensor (2x DVE mode on packed bf16)
                        xm = c1t.tile([128, 384], BF16, tag="xm", name="xm")
                        nc.vector.tensor_tensor(
                            _v(xm, 0, 128, [[1, 384]]),
                            _v(stg, 0, 128, [[1, 384]]),
                            _v(stg, 0, 128, [[1, 384]], 384),
                            ALU.max,
                        )
                        nc.vector.tensor_tensor(
                            _v(xpad2, 0, 128, [[26, 8], [1, 24]], base + tj * 8 * 26),
                            _v(xm, 0, 128, [[48, 8], [1, 24]]),
                            _v(xm, 0, 128, [[48, 8], [1, 24]], 24),
                            ALU.max,
                        )
                    else:
                        # DVE: direct 2x2 max-reduce from PSUM (relu deferred)
                        for u in range(2):
                            nc.vector.tensor_reduce(
                                _v(xpad2, 0, 128, [[26, 4], [1, 24]], base + (16 + 4 * u) * 26),
                                _v(ps, 0, 128, [[96, 4], [2, 24], [48, 2], [1, 2]], u * 512),
                                mybir.AxisListType.XY,
                                ALU.max,
                            )
                        rows = _v(xpad2, 0, 128, [[26, 8], [1, 24]], base + 16 * 26)
                        nc.vector.tensor_scalar(rows, rows, 0.0, None, ALU.max)
                # conv2 im2col chunk for samples 4m..4m+3 (gpsimd SWDGE queue)
                Q = m // 4
                if m % 4 == 0:
                    bq = b96.tile([97, 16 * 676], BF16, tag="bq", name="bq")
                    bqs[Q] = bq
                    nc.gpsimd.dma_start(_v(bq, 96, 1, [[1, 16 * 676]]), io["ones"][0:1, 0 : 16 * 676])
                bq = bqs[Q]
                for g in range(4):
                    nc.gpsimd.dma_start(
                        _v(bq, 0, 96, [[1, 676]], (4 * (m % 4) + g) * 676),
                        _v(xpad2, 32 * g, 32, [[1, 3], [1, 676]], m * 676),
                    )

            def conv2_block(b):
                bq = bqs[b // 8]
                ps = c2p.tile([128, 1024], F32, tag="ps2", name="ps2")
                for yh in range(2):
                    for h in range(2):
                        loc = (2 * b + h) - 16 * (b // 8)
                        for dy in range(3):
                            nc.tensor.matmul(
                                _v(ps, 64 * h, 64, [[1, 288]], yh * 512),
                                w2sb[0:97, dy * 64 : dy * 64 + 64],
                                _v(bq, 0, 97, [[26, 12], [1, 24]], loc * 676 + yh * 312 + dy * 26),
                                start=(dy == 0),
                                stop=(dy == 2),
                                tile_position=(0, 64 * h),
                            )
                # Act evac both banks (yh,y,xh,phase), then 2x tt max stages
                stg = c2t.tile([128, 576], BF16, tag="stg", name="stg")
                nc.scalar.activation(
                    _v(stg, 0, 128, [[144, 2], [12, 12], [1, 12], [288, 2]]),
                    _v(ps, 0, 128, [[512, 2], [24, 12], [2, 12], [1, 2]]),
                    AF.Relu,
                )
                xm = c2t.tile([128, 288], BF16, tag="xm", name="xm")
                nc.vector.tensor_tensor(
                    _v(xm, 0, 128, [[1, 288]]),
                    _v(stg, 0, 128, [[1, 288]]),
                    _v(stg, 0, 128, [[1, 288]], 288),
                    ALU.max,
                )
                nc.vector.tensor_tensor(
                    _v(xpad3, 0, 128, [[14, 12], [1, 12]], b * 198 + 15),
                    _v(xm, 0, 128, [[24, 12], [1, 12]]),
                    _v(xm, 0, 128, [[24, 12], [1, 12]], 12),
                    ALU.max,
                )
                # conv3 im2col chunk once its xpad3 sample range is complete (SP queue)
                if b == 15 or b == 31:
                    ck = b // 16
                    off = ck * 16 * 198
                    for h in range(2):
                        nc.sync.dma_start(
                            _v(bufA[h], 0, 128, [[1, 16 * 198]], off),
                            _v(xpad3, 64 * h, 64, [[1, 2], [1, 16 * 198]], off),
                        )
                        nc.sync.dma_start(
                            _v(bufB[h], 0, 64, [[1, 16 * 198]], off),
                            _v(xpad3, 64 * h, 64, [[1, 16 * 198]], off + 2),
                        )

            for m in range(16):
                conv1_chunk(m)
                if m >= 2:
                    conv2_block(2 * (m - 2))
                    conv2_block(2 * (m - 2) + 1)
            for b in range(28, 32):
                conv2_block(b)

        # FC1 weights: 4.7MB load lands during conv3
        fcw = ctx.enter_context(tc.tile_pool(name="fcw", bufs=1))
        wf1sb = fcw.tile([128, 18432], BF16)
        nc.sync.dma_start(wf1sb[:, :], io["wf1"][:, :])

        # ---------------- conv3 + FC1 (parity-pipelined) ----------------
        cur1p = ctx.enter_context(tc.tile_pool(name="cur1p", bufs=1, space="PSUM"))
        cur1 = cur1p.tile([128, 256], F32)
        with (
            tc.tile_pool(name="c3ps", bufs=4, space="PSUM") as c3p,
            tc.tile_pool(name="c3t", bufs=4) as c3t,
        ):
            for h in range(2):
                for bq_ in range(4):
                    for bj in range(4):
                        bp = bq_ * 4 + bj
                        ps = c3p.tile([128, 288], F32, tag="ps3")
                        for dy in range(3):
                            dims = [[198, 2], [14, 12], [1, 12]]
                            off = bp * 2 * 198 + dy * 14
                            nc.tensor.matmul(
                                ps[:, :], w3asb[0:128, dy * 128 : dy * 128 + 128],
                                _v(bufA[h], 0, 128, dims, off),
                                start=(dy == 0), stop=False,
                            )
                            nc.tensor.matmul(
                                ps[:, :], w3bsb[0:65, dy * 128 : dy * 128 + 128],
                                _v(bufB[h], 0, 65, dims, off),
                                start=False, stop=(dy == 2),
                            )
                        # slot of (h, bp, i) is sample 4bp+h+2i -> feat col (4bp+h+2i)*36
                        if bj < 3:
                            stg = c3t.tile([128, 288], BF16, tag="stg")
                            nc.scalar.activation(
                                _v(stg, 0, 128, [[72, 2], [6, 12], [1, 6], [144, 2]]),
                                ps[:, :], AF.Relu,
                            )
                            xm = c3t.tile([128, 144], BF16, tag="xm")
                            nc.vector.tensor_tensor(
                                _v(xm, 0, 128, [[1, 144]]),
                                _v(stg, 0, 128, [[1, 144]]),
                                _v(stg, 0, 128, [[1, 144]], 144),
                                ALU.max,
                            )
                            nc.vector.tensor_tensor(
                                _v(feat, 0, 128, [[72, 2], [6, 6], [1, 6]], (4 * bp + h) * 36),
                                _v(xm, 0, 128, [[72, 2], [12, 6], [1, 6]]),
                                _v(xm, 0, 128, [[72, 2], [12, 6], [1, 6]], 6),
                                ALU.max,
                            )
                        else:
                            # DVE direct reduce per sample (relu deferred to feat pass)
                            for i in range(2):
                                nc.vector.tensor_reduce(
                                    _v(feat, 0, 128, [[6, 6], [1, 6]], (4 * bp + h + 2 * i) * 36),
                                    _v(ps, 0, 128, [[24, 6], [2, 6], [12, 2], [1, 2]], i * 144),
                                    mybir.AxisListType.XY,
                                    ALU.max,
                                )
                # in-place relu over the DVE-reduced feat slots of this parity
                # (bp = 3,7,11,15 -> slots 4bp+h and 4bp+2+h)
                rows = _v(feat, 0, 128, [[576, 4], [72, 2], [1, 36]], (12 + h) * 36)
                nc.vector.tensor_scalar(rows, rows, 0.0, None, ALU.max)
                # FC1 for parity h: out [unit, 32 samples] at cur1 col 64g+32h
                # (samples of parity h = feat cols h, h+2, ... -> stride 72)
                for g in range(4):
                    for k in range(36):
                        nc.tensor.matmul(
                            cur1[:, 64 * g + 32 * h : 64 * g + 32 * h + 32],
                            wf1sb[:, k * 512 + g * 128 : k * 512 + g * 128 + 128],
                            _v(feat, 0, 128, [[72, 32]], k + 36 * h),
                            start=(k == 0),
                            stop=(k == 35),
                        )

        # ---------------- LIF + FC2/FC3 (layer-1 solved analytically) ----------------
        with (
            tc.tile_pool(name="cur2p", bufs=1, space="PSUM") as cur2p,
            tc.tile_pool(name="liftmp", bufs=2) as dtp,
        ):
            # s1_t straight from cur1: v=(v+c)/2, th=1, hard reset =>
            # s1_t1 = [c>=2]; s1_t2 = [c>=4/3]; s1_t3 = [c>=8/7] - [c>=4/3] + [c>=2]
            c43 = float(np.float32(4.0) / np.float32(3.0))
            c87 = float(np.float32(8.0) / np.float32(7.0))
            t87 = dtp.tile([128, 256], BF16, tag="t87")
            nc.vector.tensor_scalar(_v(s1_all, 0, 128, [[1, 256]]), cur1[:, :], 2.0, None, ALU.is_ge)
            nc.vector.tensor_scalar(_v(s1_all, 0, 128, [[1, 256]], 256), cur1[:, :], c43, None, ALU.is_ge)
            nc.vector.tensor_scalar(t87[:, :], cur1[:, :], c87, None, ALU.is_ge)
            nc.vector.tensor_tensor(
                _v(s1_all, 0, 128, [[1, 256]], 512), t87[:, :],
                _v(s1_all, 0, 128, [[1, 256]], 256), ALU.subtract,
            )
            nc.vector.tensor_tensor(
                _v(s1_all, 0, 128, [[1, 256]], 512),
                _v(s1_all, 0, 128, [[1, 256]], 512),
                _v(s1_all, 0, 128, [[1, 256]]), ALU.add,
            )

            # FC2 for all 3 timesteps at once: N=192 (wf2 pre-scaled 0.5 on host)
            cur2 = cur2p.tile([128, 192], F32, tag="cur2")
            for g in range(4):
                nc.tensor.matmul(
                    cur2[:, :], wf2sb[:, g * 128 : g * 128 + 128],
                    _v(s1_all, 0, 128, [[256, 3], [1, 64]], 64 * g),
                    start=(g == 0), stop=(g == 3),
                )

            cur3 = cur2p.tile([5, 192], F32, tag="cur3")

            def lif2(t):
                # v2 <- v2*0.5 + cur2_half ; s2 = (v2 >= 1) ; v2 <- 0 where s2
                nc.vector.scalar_tensor_tensor(
                    v2[:, :], v2[:, :], 0.5, cur2[:, 64 * t : 64 * t + 64], ALU.mult, ALU.add)
                s2t = _v(s2_all, 0, 128, [[1, 64]], 64 * t)
                nc.vector.tensor_scalar(s2t, v2[:, :], 1.0, None, ALU.is_ge)
                nc.vector.copy_predicated(v2[:, :], s2t.bitcast(mybir.dt.uint16), zeros[:, :])
                nc.tensor.matmul(
                    cur3[0:5, 64 * t : 64 * t + 64], wf3sb[0:128, 0:5],
                    _v(s2_all, 0, 128, [[1, 64]], 64 * t),
                    start=True, stop=True,
                )

            def lif3(t):
                nc.vector.scalar_tensor_tensor(
                    v3[:, :], v3[:, :], 0.5, cur3[0:5, 64 * t : 64 * t + 64], ALU.mult, ALU.add)
                s3 = dtp.tile([5, 64], F32, tag="s3")
                nc.vector.tensor_scalar(s3[:, :], v3[:, :], 1.0, None, ALU.is_ge)
                nc.vector.copy_predicated(v3[:, :], s3[:, :].bitcast(mybir.dt.uint32), zeros[0:5, 0:64])
                nc.vector.tensor_tensor(acc[:, :], acc[:, :], s3[:, :], ALU.add)

            lif2(0)
            lif2(1)
            lif3(0)
            lif2(2)
            lif3(1)
            lif3(2)

            # acc/3 for acc in {0,1,2,3}: mult by fp32(1/3) matches true division except acc=3
            # (3*0.33333334 = 1.0000001) -> clamp with min(., 1.0) for exactness.
            nc.vector.tensor_scalar(acc[:, :], acc[:, :], float(np.float32(1.0) / np.float32(3.0)), 1.0, ALU.mult, ALU.min)
            # acc col (32h + j) holds sample 2j+h -> un-permute on the way out
            for h in range(2):
                nc.sync.dma_start(
                    _dv(io["out"], h, [[64, 5], [2, 32]]),
                    _v(acc, 0, 5, [[1, 32]], 32 * h),
                )


def _build():
    nc = bacc.Bacc("TRN2", target_bir_lowering=False, debug=False, enable_asserts=True)
    io = {}

    def inp(name, shape, dt):
        io[name] = nc.dram_tensor(name, shape, dt, kind="ExternalInput").ap()

    inp("imc", [109, 40000], BF16)
    inp("w1l", [109, 128], BF16)
    inp("w2l", [97, 192], BF16)
    inp("w3a", [128, 384], BF16)
    inp("w3b", [65, 384], BF16)
    inp("wf1", [128, 18432], BF16)
    inp("wf2", [128, 512], BF16)
    inp("wf3", [128, 5], BF16)
    inp("ones", [1, 10816], BF16)
    io["out"] = nc.dram_tensor("out", [5, 64], F32, kind="ExternalOutput").ap()

    with tile.TileContext(nc) as tc:
        _emit(tc, io)
    nc.compile()
    return nc


def _fake_quant(w):
    w = np.asarray(w, np.float32)
    scale = np.float32(np.max(np.abs(w)) / np.float32(127.0))
    wq = np.clip(np.round(w / scale), -127.0, 127.0).astype(np.float32) * scale
    return wq.astype(np.float32)


def _bf16(a):
    return np.asarray(a, np.float32).astype(ml_dtypes.bfloat16)


def _prep_weights(conv1_w, conv1_b, conv2_w, conv2_b, conv3_w, conv3_b, W1, W2, W3):
    c1 = np.asarray(conv1_w, np.float32)  # [32, 3, 3, 3]
    c2 = np.asarray(conv2_w, np.float32)  # [64, 32, 3, 3]
    c3 = np.asarray(conv3_w, np.float32)  # [128, 64, 3, 3]

    # conv1 block-diagonal: rows 27g..27g+26 = taps of group g -> cols 32g..32g+31;
    # row 108 = bias (tiled 4x over the 4 col groups).
    w1l = np.zeros((109, 128), np.float32)
    wk = c1.transpose(2, 3, 1, 0).reshape(27, 32)  # [(dy,dx,c), m]
    for q in range(4):
        w1l[27 * q : 27 * q + 27, 32 * q : 32 * q + 32] = wk
    w1l[108, :] = np.tile(np.asarray(conv1_b, np.float32), 4)

    w2l = np.zeros((97, 192), np.float32)
    w2l[0:96] = c2.transpose(1, 3, 2, 0).reshape(96, 192)  # [(c,dx), (dy,m)]
    w2l[96, 0:64] = np.asarray(conv2_b, np.float32)        # bias rides the dy=0 block

    w3x = c3.transpose(1, 3, 2, 0)  # [c, dx, dy, m]
    w3a = w3x[:, 0:2].reshape(128, 384)
    w3b = np.zeros((65, 384), np.float32)
    w3b[0:64] = w3x[:, 2].reshape(64, 384)
    w3b[64, 0:128] = np.asarray(conv3_b, np.float32)       # bias rides the dy=0 block

    W1q = _fake_quant(W1)  # [512, 4608]
    W2q = _fake_quant(W2)  # [128, 512]
    W3q = _fake_quant(W3)  # [5, 128]

    # [c, k*512 + u] = W1q[u, c*36 + k]  (FC1 weight-stationary: out [unit, sample])
    wf1 = W1q.reshape(512, 128, 36).transpose(1, 2, 0).reshape(128, 36 * 512)
    # FC2/FC3 pre-scaled by 0.5: LIF v-update becomes v*0.5 + cur_half in one op
    wf2 = 0.5 * W2q.T.reshape(4, 128, 128).transpose(1, 0, 2).reshape(128, 512)
    wf3 = 0.5 * W3q.T  # [128, 5]

    return {
        "w1l": _bf16(w1l),
        "w2l": _bf16(w2l),
        "w3a": _bf16(w3a),
        "w3b": _bf16(w3b),
        "wf1": _bf16(wf1),
        "wf2": _bf16(wf2),
        "wf3": _bf16(wf3),
        "ones": _bf16(np.ones((1, 10816), np.float32)),
    }


_NC = None
LAST_RESULTS = None


def kernel(x, conv1_w, conv1_b, conv2_w, conv2_b, conv3_w, conv3_b, W1, W2, W3, _trace=False):
    global _NC, LAST_RESULTS
    if _NC is None:
        _NC = _build()

    wmap = _prep_weights(conv1_w, conv1_b, conv2_w, conv2_b, conv3_w, conv3_b, W1, W2, W3)

    # host-side im2col for conv1, chunk-ordered: chunk m = samples 4m..4m+3,
    # imc[27g + dy*9 + dx*3 + c, m*2500 + p] = xpad[4m+g, c, p + dy*50 + dx]
    x = np.asarray(x, np.float32)
    xp = np.zeros((512, 3, 50, 50), np.float32)
    xp[:, :, 1:49, 1:49] = x
    xf = np.zeros((512, 3, 2604), np.float32)
    xf[:, :, :2500] = xp.reshape(512, 3, 2500)

    in_maps = []
    for i in range(NCORES):
        S = xf[B * i : B * (i + 1)].reshape(16, 4, 3, 2604)
        A = np.zeros((109, 40000), np.float32)
        for dy in range(3):
            for dx in range(3):
                blk = S[:, :, :, dy * 50 + dx : dy * 50 + dx + 2500]  # [m, g, c, p]
                for g in range(4):
                    r = 27 * g + dy * 9 + dx * 3
                    A[r : r + 3] = blk[:, g].transpose(1, 0, 2).reshape(3, 40000)
        A[108] = 1.0
        in_maps.append({"imc": _bf16(A), **wmap})

    from concourse.bass_utils import run_bass_kernel_spmd

    res = run_bass_kernel_spmd(_NC, in_maps, core_ids=list(range(NCORES)), trace=_trace)
    LAST_RESULTS = res
    out = np.concatenate([np.asarray(res.results[i]["out"]).T for i in range(NCORES)], axis=0)
    return np.ascontiguousarray(out.astype(np.float32))


# revision 14
# speedup vs baseline: 1.1951x; 1.0439x over previous
"""Trainium2 Bass kernel for ConvFCNet (3x conv+pool -> int8-fakequant FC + LIF SNN head).

Data-parallel over 8 NeuronCores: batch 512 -> 64 samples/core, weights replicated.

v1 rework (from 156us baseline): the PE queue is kept continuously fed so the
tensor engine stays at full p-state and is the binding resource (~89us of
matmul work):
  - conv1 im2col is built on the HOST in per-chunk order (chunk m = samples
    4m..4m+3 via the block-diagonal group trick), DMAed in 16 fine-grained
    chunks so the first matmul starts at ~4us instead of 11us.
  - conv2 blocks are emitted interleaved into the conv1 chunk loop (lag 2),
    so conv2 matmuls run while conv1 pooling drains instead of after it.
  - pooling max stages use tensor_tensor(max) (2x DVE perf mode for packed
    bf16) instead of scalar_tensor_tensor (no perf mode), and PSUM tiles span
    2 banks so one Act evacuation covers 2 matmul tiles.
  - LIF layer-1 is solved analytically across the 3 timesteps straight from
    the cur1 PSUM (s1_t thresholds 2, 4/3, 8/7 on cur1), FC2 runs all 3
    timesteps in one matmul set (N=192), and FC2/FC3 weights are pre-scaled
    by 0.5 on the host so the LIF v-update is a single scalar_tensor_tensor.
"""

import numpy as np
import ml_dtypes

import concourse.bass as bass
import concourse.bacc as bacc
import concourse.tile as tile
import concourse.mybir as mybir

AF = mybir.ActivationFunctionType
ALU = mybir.AluOpType
BF16 = mybir.dt.bfloat16
F32 = mybir.dt.float32

NCORES = 8
B = 64  # samples per core


def _v(ap, p0, npart, dims, off=0):
    """View into an SBUF/PSUM tile AP: partition slice [p0, p0+npart) + custom free dims."""
    pitch = ap.ap[0][0]
    return bass.AP(
        tensor=ap.tensor,
        offset=ap.offset + p0 * pitch + off,
        ap=[[pitch, npart]] + [list(d) for d in dims],
    )


def _dv(ap, off, dims):
    """View into a DRAM tensor AP with custom dims."""
    return bass.AP(tensor=ap.tensor, offset=ap.offset + off, ap=[list(d) for d in dims])


def _emit(tc, io):
    nc = tc.nc
    from contextlib import ExitStack

    with ExitStack() as ctx:
        # ---------------- persistent buffers + weights ----------------
        wp = ctx.enter_context(tc.tile_pool(name="wts", bufs=1))
        w1sb = wp.tile([109, 128], BF16)
        nc.gpsimd.dma_start(w1sb[:, :], io["w1l"][:, :])
        w2sb = wp.tile([97, 192], BF16)
        nc.gpsimd.dma_start(w2sb[:, :], io["w2l"][:, :])
        w3asb = wp.tile([128, 384], BF16)
        nc.gpsimd.dma_start(w3asb[:, :], io["w3a"][:, :])
        w3bsb = wp.tile([65, 384], BF16)
        nc.gpsimd.dma_start(w3bsb[:, :], io["w3b"][:, :])
        wf2sb = wp.tile([128, 512], BF16)
        nc.gpsimd.dma_start(wf2sb[:, :], io["wf2"][:, :])
        wf3sb = wp.tile([128, 5], BF16)
        nc.gpsimd.dma_start(wf3sb[:, :], io["wf3"][:, :])

        # preload the Relu activation table while the head DMAs run
        scr = wp.tile([1, 8], BF16)
        nc.scalar.activation(_v(scr, 0, 1, [[1, 8]]), _v(w1sb, 0, 1, [[1, 8]]), AF.Relu)

        mp = ctx.enter_context(tc.tile_pool(name="main", bufs=1))
        # conv1 pooled output, padded 26x26; partition 32g+c = sample 4m+g at col m*676
        xpad2 = mp.tile([128, 16 * 676 + 4], BF16)
        for dims, off in [
            ([[676, 16], [1, 26]], 0),        # top row
            ([[676, 16], [1, 26]], 650),      # bottom row
            ([[676, 16], [26, 26]], 0),       # left col
            ([[676, 16], [26, 26]], 25),      # right col
            ([[1, 4]], 16 * 676),             # tail pad (im2col dx over-read)
        ]:
            nc.gpsimd.memset(_v(xpad2, 0, 128, dims, off), 0.0)
        # conv2 pooled output, padded 14x14, partition 64h+c holds samples of parity h
        xpad3 = mp.tile([128, 32 * 198 + 4], BF16)
        for dims, off in [
            ([[198, 32], [1, 14]], 0),        # top row
            ([[198, 32], [1, 14]], 182),      # bottom row
            ([[198, 32], [14, 14]], 0),       # left col
            ([[198, 32], [14, 14]], 13),      # right col
            ([[1, 4]], 32 * 198),             # tail pad (im2col dx over-read)
            ([[198, 32], [1, 2]], 196),       # per-sample slack (pitch 198 vs 196)
        ]:
            nc.gpsimd.memset(_v(xpad3, 0, 128, dims, off), 0.0)
        # conv3 pooled output (features): [128c, sample*36 + hw]
        feat = mp.tile([128, B * 36], BF16)

        # LIF state (hoisted memsets run during the DMA head)
        lifp = ctx.enter_context(tc.tile_pool(name="lif", bufs=1))
        zeros = lifp.tile([128, 64], F32)
        nc.gpsimd.memset(zeros[:, :], 0.0)
        v2 = lifp.tile([128, 64], F32)
        nc.gpsimd.memset(v2[:, :], 0.0)
        v3 = lifp.tile([5, 64], F32)
        nc.gpsimd.memset(v3[:, :], 0.0)
        acc = lifp.tile([5, 64], F32)
        nc.gpsimd.memset(acc[:, :], 0.0)
        s1_all = lifp.tile([128, 768], BF16)   # [t*256 + cur1-col]
        s2_all = lifp.tile([128, 192], BF16)   # [t*64 + sample-col]

        # conv3 im2col buffers (row 64 of B = bias row)
        c3b = ctx.enter_context(tc.tile_pool(name="c3buf", bufs=1))
        bufA = [c3b.tile([128, 32 * 198 + 4], BF16, name=f"bufA{h}") for h in range(2)]
        bufB = [c3b.tile([65, 32 * 198 + 4], BF16, name=f"bufB{h}") for h in range(2)]
        for h in range(2):
            nc.sync.dma_start(_v(bufB[h], 64, 1, [[1, 32 * 198 + 4]]), io["ones"][0:1, 0 : 32 * 198 + 4])

        # FC1 weights: loaded in 4 chunks spread across the conv1/conv2 window
        # (a single 13us DMA would block the serialized DMA engines)
        fcw = ctx.enter_context(tc.tile_pool(name="fcw", bufs=1))
        wf1sb = fcw.tile([128, 18432], BF16)

        # conv2 im2col quarters (96 rows = 32c x 3dx, row 96 = bias row), scoped
        b96 = ctx.enter_context(tc.tile_pool(name="b96", bufs=2))
        bqs = {}

        # ---------------- conv1 + conv2 + conv3 (interleaved, PE stays fed) ----------------
        with (
            tc.tile_pool(name="c2ps", bufs=2, space="PSUM") as c2p,
            tc.tile_pool(name="c2t", bufs=3) as c2t,
        ):
            def conv1_chunk(m, c1i, c1p, c1t):
                imct = c1i.tile([109, 2500], BF16, tag="imc", name="imc")
                nc.sync.dma_start(
                    _v(imct, 0, 109, [[1, 2500]]),
                    _dv(io["imc"], m * 2500, [[40000, 109], [1, 2500]]),
                )
                base = m * 676 + 27
                for tj in range(3):
                    ps = c1p.tile([128, 1024], F32, tag="ps1", name="ps1")
                    for u in range(2):
                        nc.tensor.matmul(
                            _v(ps, 0, 128, [[1, 384]], u * 512),
                            _v(w1sb, 0, 109, [[1, 128]]),
                            _v(imct, 0, 109, [[50, 8], [1, 48]], (tj * 2 + u) * 400),
                            start=True,
                            stop=True,
                        )
                    if tj < 2:
                        # Act evac: relu+copy both banks, x-deinterleaved (u,y,xh,phase)
                        stg = c1t.tile([128, 768], BF16, tag="stg", name="stg")
                        nc.scalar.activation(
                            _v(stg, 0, 128, [[192, 2], [24, 8], [1, 24], [384, 2]]),
                            _v(ps, 0, 128, [[512, 2], [48, 8], [2, 24], [1, 2]]),
                            AF.Relu,
                        )
                        # max stages as tensor_tensor (2x DVE mode on packed bf16)
                        xm = c1t.tile([128, 384], BF16, tag="xm", name="xm")
                        nc.vector.tensor_tensor(
                            _v(xm, 0, 128, [[1, 384]]),
                            _v(stg, 0, 128, [[1, 384]]),
                            _v(stg, 0, 128, [[1, 384]], 384),
                            ALU.max,
                        )
                        nc.vector.tensor_tensor(
                            _v(xpad2, 0, 128, [[26, 8], [1, 24]], base + tj * 8 * 26),
                            _v(xm, 0, 128, [[48, 8], [1, 24]]),
                            _v(xm, 0, 128, [[48, 8], [1, 24]], 24),
                            ALU.max,
                        )
                    else:
                        # DVE: direct 2x2 max-reduce from PSUM (relu deferred)
                        for u in range(2):
                            nc.vector.tensor_reduce(
                                _v(xpad2, 0, 128, [[26, 4], [1, 24]], base + (16 + 4 * u) * 26),
                                _v(ps, 0, 128, [[96, 4], [2, 24], [48, 2], [1, 2]], u * 512),
                                mybir.AxisListType.XY,
                                ALU.max,
                            )
                        rows = _v(xpad2, 0, 128, [[26, 8], [1, 24]], base + 16 * 26)
                        nc.vector.tensor_scalar(rows, rows, 0.0, None, ALU.max)
                # conv2 im2col: batched per quarter (16 samples) to amortize SWDGE
                # descriptor-generation cost on the gpsimd queue
                Q = m // 4
                if m % 4 == 0:
                    bq = b96.tile([97, 16 * 676], BF16, tag="bq", name="bq")
                    bqs[Q] = bq
                    nc.gpsimd.dma_start(_v(bq, 96, 1, [[1, 16 * 676]]), io["ones"][0:1, 0 : 16 * 676])
                if m % 4 == 3:
                    # quarter col layout is (g, chunk): sample 16Q+4c+g at col (4g+c)*676
                    bq = bqs[Q]
                    for g in range(4):
                        nc.gpsimd.dma_start(
                            _v(bq, 0, 96, [[1, 2704]], g * 4 * 676),
                            _v(xpad2, 32 * g, 32, [[1, 3], [1, 2704]], Q * 4 * 676),
                        )
                    # a 4.6KB-per-partition slice of the FC1 weights rides along here
                    nc.gpsimd.dma_start(
                        wf1sb[:, Q * 4608 : (Q + 1) * 4608],
                        _dv(io["wf1"], Q * 4608, [[18432, 128], [1, 4608]]),
                    )

            def conv2_block(b):
                bq = bqs[b // 8]
                ps = c2p.tile([128, 1024], F32, tag="ps2", name="ps2")
                for yh in range(2):
                    for h in range(2):
                        s = 2 * b + h
                        loc = 4 * (s % 4) + (s // 4 - 4 * (b // 8))
                        for dy in range(3):
                            nc.tensor.matmul(
                                _v(ps, 64 * h, 64, [[1, 288]], yh * 512),
                                w2sb[0:97, dy * 64 : dy * 64 + 64],
                                _v(bq, 0, 97, [[26, 12], [1, 24]], loc * 676 + yh * 312 + dy * 26),
                                start=(dy == 0),
                                stop=(dy == 2),
                                tile_position=(0, 64 * h),
                            )
                # Act evac both banks (yh,y,xh,phase), then 2x tt max stages
                stg = c2t.tile([128, 576], BF16, tag="stg", name="stg")
                nc.scalar.activation(
                    _v(stg, 0, 128, [[144, 2], [12, 12], [1, 12], [288, 2]]),
                    _v(ps, 0, 128, [[512, 2], [24, 12], [2, 12], [1, 2]]),
                    AF.Relu,
                )
                xm = c2t.tile([128, 288], BF16, tag="xm", name="xm")
                nc.vector.tensor_tensor(
                    _v(xm, 0, 128, [[1, 288]]),
                    _v(stg, 0, 128, [[1, 288]]),
                    _v(stg, 0, 128, [[1, 288]], 288),
                    ALU.max,
                )
                nc.vector.tensor_tensor(
                    _v(xpad3, 0, 128, [[14, 12], [1, 12]], b * 198 + 15),
                    _v(xm, 0, 128, [[24, 12], [1, 12]]),
                    _v(xm, 0, 128, [[24, 12], [1, 12]], 12),
                    ALU.max,
                )
                # conv3 im2col chunk once its xpad3 sample range is complete
                if b == 15 or b == 31:
                    ck = b // 16
                    off = ck * 16 * 198
                    for h in range(2):
                        nc.gpsimd.dma_start(
                            _v(bufA[h], 0, 128, [[1, 16 * 198]], off),
                            _v(xpad3, 64 * h, 64, [[1, 2], [1, 16 * 198]], off),
                        )
                        nc.gpsimd.dma_start(
                            _v(bufB[h], 0, 64, [[1, 16 * 198]], off),
                            _v(xpad3, 64 * h, 64, [[1, 16 * 198]], off + 2),
                        )

            def conv3_unit(h, bp):
                # c3p/c3t are opened after the conv1 PSUM pool closes (bank budget)
                bj = bp % 4
                ps = c3p.tile([128, 288], F32, tag="ps3", name="ps3")
                for dy in range(3):
                    dims = [[198, 2], [14, 12], [1, 12]]
                    off = bp * 2 * 198 + dy * 14
                    nc.tensor.matmul(
                        ps[:, :], w3asb[0:128, dy * 128 : dy * 128 + 128],
                        _v(bufA[h], 0, 128, dims, off),
                        start=(dy == 0), stop=False,
                    )
                    nc.tensor.matmul(
                        ps[:, :], w3bsb[0:65, dy * 128 : dy * 128 + 128],
                        _v(bufB[h], 0, 65, dims, off),
                        start=False, stop=(dy == 2),
                    )
                # slot of (h, bp, i) is sample 4bp+h+2i -> feat col (4bp+h+2i)*36
                if bj < 3:
                    stg = c3t.tile([128, 288], BF16, tag="stg", name="stg")
                    nc.scalar.activation(
                        _v(stg, 0, 128, [[72, 2], [6, 12], [1, 6], [144, 2]]),
                        ps[:, :], AF.Relu,
                    )
                    xm = c3t.tile([128, 144], BF16, tag="xm", name="xm")
                    nc.vector.tensor_tensor(
                        _v(xm, 0, 128, [[1, 144]]),
                        _v(stg, 0, 128, [[1, 144]]),
                        _v(stg, 0, 128, [[1, 144]], 144),
                        ALU.max,
                    )
                    nc.vector.tensor_tensor(
                        _v(feat, 0, 128, [[72, 2], [6, 6], [1, 6]], (4 * bp + h) * 36),
                        _v(xm, 0, 128, [[72, 2], [12, 6], [1, 6]]),
                        _v(xm, 0, 128, [[72, 2], [12, 6], [1, 6]], 6),
                        ALU.max,
                    )
                else:
                    # DVE direct reduce per sample (relu deferred to feat pass)
                    for i in range(2):
                        nc.vector.tensor_reduce(
                            _v(feat, 0, 128, [[6, 6], [1, 6]], (4 * bp + h + 2 * i) * 36),
                            _v(ps, 0, 128, [[24, 6], [2, 6], [12, 2], [1, 2]], i * 144),
                            mybir.AxisListType.XY,
                            ALU.max,
                        )

            with (
                tc.tile_pool(name="c1imc", bufs=4) as c1i,
                tc.tile_pool(name="c1ps", bufs=2, space="PSUM") as c1p,
                tc.tile_pool(name="c1t", bufs=3) as c1t,
            ):
                for m in range(16):
                    conv1_chunk(m, c1i, c1p, c1t)
                    if m >= 4:
                        conv2_block(2 * (m - 4))
                        conv2_block(2 * (m - 4) + 1)
            for b in range(24, 32):
                conv2_block(b)

        # ---------------- conv3 + FC1 (parity-pipelined) ----------------
        c3p = ctx.enter_context(tc.tile_pool(name="c3ps", bufs=3, space="PSUM"))
        c3t = ctx.enter_context(tc.tile_pool(name="c3t", bufs=4))
        cur1p = ctx.enter_context(tc.tile_pool(name="cur1p", bufs=1, space="PSUM"))
        cur1 = cur1p.tile([128, 256], F32)
        # ck0 units first (their im2col chunk landed at b=15); ck1's chunk
        # (emitted at b=31) transfers while these run
        for bp in range(8):
            for h in range(2):
                conv3_unit(h, bp)
        for h in range(2):
            for bp in range(8, 16):
                conv3_unit(h, bp)
            # in-place relu over the DVE-reduced feat slots of this parity
            # (bp = 3,7,11,15 -> slots 4bp+h and 4bp+2+h)
            rows = _v(feat, 0, 128, [[576, 4], [72, 2], [1, 36]], (12 + h) * 36)
            nc.vector.tensor_scalar(rows, rows, 0.0, None, ALU.max)
            # FC1 for parity h: out [unit, 32 samples] at cur1 col 64g+32h
            # (samples of parity h = feat cols h, h+2, ... -> stride 72)
            for g in range(4):
                for k in range(36):
                    nc.tensor.matmul(
                        cur1[:, 64 * g + 32 * h : 64 * g + 32 * h + 32],
                        wf1sb[:, k * 512 + g * 128 : k * 512 + g * 128 + 128],
                        _v(feat, 0, 128, [[72, 32]], k + 36 * h),
                        start=(k == 0),
                        stop=(k == 35),
                    )

        # ---------------- LIF + FC2/FC3 (layer-1 solved analytically) ----------------
        with (
            tc.tile_pool(name="cur2p", bufs=1, space="PSUM") as cur2p,
            tc.tile_pool(name="liftmp", bufs=2) as dtp,
        ):
            # s1_t straight from cur1: v=(v+c)/2, th=1, hard reset =>
            # s1_t1 = [c>=2]; s1_t2 = [c>=4/3]; s1_t3 = [c>=8/7] - [c>=4/3] + [c>=2]
            c43 = float(np.float32(4.0) / np.float32(3.0))
            c87 = float(np.float32(8.0) / np.float32(7.0))
            t87 = dtp.tile([128, 256], BF16, tag="t87")
            nc.vector.tensor_scalar(_v(s1_all, 0, 128, [[1, 256]]), cur1[:, :], 2.0, None, ALU.is_ge)
            nc.vector.tensor_scalar(_v(s1_all, 0, 128, [[1, 256]], 256), cur1[:, :], c43, None, ALU.is_ge)
            nc.vector.tensor_scalar(t87[:, :], cur1[:, :], c87, None, ALU.is_ge)
            nc.vector.tensor_tensor(
                _v(s1_all, 0, 128, [[1, 256]], 512), t87[:, :],
                _v(s1_all, 0, 128, [[1, 256]], 256), ALU.subtract,
            )
            nc.vector.tensor_tensor(
                _v(s1_all, 0, 128, [[1, 256]], 512),
                _v(s1_all, 0, 128, [[1, 256]], 512),
                _v(s1_all, 0, 128, [[1, 256]]), ALU.add,
            )

            # FC2 for all 3 timesteps at once: N=192 (wf2 pre-scaled 0.5 on host)
            cur2 = cur2p.tile([128, 192], F32, tag="cur2")
            for g in range(4):
                nc.tensor.matmul(
                    cur2[:, :], wf2sb[:, g * 128 : g * 128 + 128],
                    _v(s1_all, 0, 128, [[256, 3], [1, 64]], 64 * g),
                    start=(g == 0), stop=(g == 3),
                )

            cur3 = cur2p.tile([5, 192], F32, tag="cur3")

            def lif2(t):
                # v2 <- v2*0.5 + cur2_half ; s2 = (v2 >= 1) ; v2 <- 0 where s2
                nc.vector.scalar_tensor_tensor(
                    v2[:, :], v2[:, :], 0.5, cur2[:, 64 * t : 64 * t + 64], ALU.mult, ALU.add)
                s2t = _v(s2_all, 0, 128, [[1, 64]], 64 * t)
                nc.vector.tensor_scalar(s2t, v2[:, :], 1.0, None, ALU.is_ge)
                nc.vector.copy_predicated(v2[:, :], s2t.bitcast(mybir.dt.uint16), zeros[:, :])
                nc.tensor.matmul(
                    cur3[0:5, 64 * t : 64 * t + 64], wf3sb[0:128, 0:5],
                    _v(s2_all, 0, 128, [[1, 64]], 64 * t),
                    start=True, stop=True,
                )

            def lif3(t):
                nc.vector.scalar_tensor_tensor(
                    v3[:, :], v3[:, :], 0.5, cur3[0:5, 64 * t : 64 * t + 64], ALU.mult, ALU.add)
                s3 = dtp.tile([5, 64], F32, tag="s3")
                nc.vector.tensor_scalar(s3[:, :], v3[:, :], 1.0, None, ALU.is_ge)
                nc.vector.copy_predicated(v3[:, :], s3[:, :].bitcast(mybir.dt.uint32), zeros[0:5, 0:64])
                nc.vector.tensor_tensor(acc[:, :], acc[:, :], s3[:, :], ALU.add)

            lif2(0)
            lif2(1)
            lif3(0)
            lif2(2)
            lif3(1)
            lif3(2)

            # acc/3 for acc in {0,1,2,3}: mult by fp32(1/3) matches true division except acc=3
            # (3*0.33333334 = 1.0000001) -> clamp with min(., 1.0) for exactness.
            nc.vector.tensor_scalar(acc[:, :], acc[:, :], float(np.float32(1.0) / np.float32(3.0)), 1.0, ALU.mult, ALU.min)
            # acc col (32h + j) holds sample 2j+h -> un-permute on the way out
            for h in range(2):
                nc.sync.dma_start(
                    _dv(io["out"], h, [[64, 5], [2, 32]]),
                    _v(acc, 0, 5, [[1, 32]], 32 * h),
                )


def _build():
    nc = bacc.Bacc("TRN2", target_bir_lowering=False, debug=False, enable_asserts=True)
    io = {}

    def inp(name, shape, dt):
        io[name] = nc.dram_tensor(name, shape, dt, kind="ExternalInput").ap()

    inp("imc", [109, 40000], BF16)
    inp("w1l", [109, 128], BF16)
    inp("w2l", [97, 192], BF16)
    inp("w3a", [128, 384], BF16)
    inp("w3b", [65, 384], BF16)
    inp("wf1", [128, 18432], BF16)
    inp("wf2", [128, 512], BF16)
    inp("wf3", [128, 5], BF16)
    inp("ones", [1, 10816], BF16)
    io["out"] = nc.dram_tensor("out", [5, 64], F32, kind="ExternalOutput").ap()

    with tile.TileContext(nc) as tc:
        _emit(tc, io)
    nc.compile()
    return nc


def _fake_quant(w):
    w = np.asarray(w, np.float32)
    scale = np.float32(np.max(np.abs(w)) / np.float32(127.0))
    wq = np.clip(np.round(w / scale), -127.0, 127.0).astype(np.float32) * scale
    return wq.astype(np.float32)


def _bf16(a):
    return np.asarray(a, np.float32).astype(ml_dtypes.bfloat16)


def _prep_weights(conv1_w, conv1_b, conv2_w, conv2_b, conv3_w, conv3_b, W1, W2, W3):
    c1 = np.asarray(conv1_w, np.float32)  # [32, 3, 3, 3]
    c2 = np.asarray(conv2_w, np.float32)  # [64, 32, 3, 3]
    c3 = np.asarray(conv3_w, np.float32)  # [128, 64, 3, 3]

    # conv1 block-diagonal: rows 27g..27g+26 = taps of group g -> cols 32g..32g+31;
    # row 108 = bias (tiled 4x over the 4 col groups).
    w1l = np.zeros((109, 128), np.float32)
    wk = c1.transpose(2, 3, 1, 0).reshape(27, 32)  # [(dy,dx,c), m]
    for q in range(4):
        w1l[27 * q : 27 * q + 27, 32 * q : 32 * q + 32] = wk
    w1l[108, :] = np.tile(np.asarray(conv1_b, np.float32), 4)

    w2l = np.zeros((97, 192), np.float32)
    w2l[0:96] = c2.transpose(1, 3, 2, 0).reshape(96, 192)  # [(c,dx), (dy,m)]
    w2l[96, 0:64] = np.asarray(conv2_b, np.float32)        # bias rides the dy=0 block

    w3x = c3.transpose(1, 3, 2, 0)  # [c, dx, dy, m]
    w3a = w3x[:, 0:2].reshape(128, 384)
    w3b = np.zeros((65, 384), np.float32)
    w3b[0:64] = w3x[:, 2].reshape(64, 384)
    w3b[64, 0:128] = np.asarray(conv3_b, np.float32)       # bias rides the dy=0 block

    W1q = _fake_quant(W1)  # [512, 4608]
    W2q = _fake_quant(W2)  # [128, 512]
    W3q = _fake_quant(W3)  # [5, 128]

    # [c, k*512 + u] = W1q[u, c*36 + k]  (FC1 weight-stationary: out [unit, sample])
    wf1 = W1q.reshape(512, 128, 36).transpose(1, 2, 0).reshape(128, 36 * 512)
    # FC2/FC3 pre-scaled by 0.5: LIF v-update becomes v*0.5 + cur_half in one op
    wf2 = 0.5 * W2q.T.reshape(4, 128, 128).transpose(1, 0, 2).reshape(128, 512)
    wf3 = 0.5 * W3q.T  # [128, 5]

    return {
        "w1l": _bf16(w1l),
        "w2l": _bf16(w2l),
        "w3a": _bf16(w3a),
        "w3b": _bf16(w3b),
        "wf1": _bf16(wf1),
        "wf2": _bf16(wf2),
        "wf3": _bf16(wf3),
        "ones": _bf16(np.ones((1, 10816), np.float32)),
    }


_NC = None
LAST_RESULTS = None


def kernel(x, conv1_w, conv1_b, conv2_w, conv2_b, conv3_w, conv3_b, W1, W2, W3, _trace=False):
    global _NC, LAST_RESULTS
    if _NC is None:
        _NC = _build()

    wmap = _prep_weights(conv1_w, conv1_b, conv2_w, conv2_b, conv3_w, conv3_b, W1, W2, W3)

    # host-side im2col for conv1, chunk-ordered: chunk m = samples 4m..4m+3,
    # imc[27g + dy*9 + dx*3 + c, m*2500 + p] = xpad[4m+g, c, p + dy*50 + dx]
    x = np.asarray(x, np.float32)
    xp = np.zeros((512, 3, 50, 50), np.float32)
    xp[:, :, 1:49, 1:49] = x
    xf = np.zeros((512, 3, 2604), np.float32)
    xf[:, :, :2500] = xp.reshape(512, 3, 2500)

    in_maps = []
    for i in range(NCORES):
        S = xf[B * i : B * (i + 1)].reshape(16, 4, 3, 2604)
        A = np.zeros((109, 40000), np.float32)
        for dy in range(3):
            for dx in range(3):
                blk = S[:, :, :, dy * 50 + dx : dy * 50 + dx + 2500]  # [m, g, c, p]
                for g in range(4):
                    r = 27 * g + dy * 9 + dx * 3
                    A[r : r + 3] = blk[:, g].transpose(1, 0, 2).reshape(3, 40000)
        A[108] = 1.0
        in_maps.append({"imc": _bf16(A), **wmap})

    from concourse.bass_utils import run_bass_kernel_spmd

    res = run_bass_kernel_spmd(_NC, in_maps, core_ids=list(range(NCORES)), trace=_trace)
    LAST_RESULTS = res
    out = np.concatenate([np.asarray(res.results[i]["out"]).T for i in range(NCORES)], axis=0)
    return np.ascontiguousarray(out.astype(np.float32))


# revision 23
# speedup vs baseline: 1.2037x; 1.0073x over previous
"""Trainium2 Bass kernel for ConvFCNet (3x conv+pool -> int8-fakequant FC + LIF SNN head).

Data-parallel over 8 NeuronCores: batch 512 -> 64 samples/core, weights replicated.

v1 rework (from 156us baseline): the PE queue is kept continuously fed so the
tensor engine stays at full p-state and is the binding resource (~89us of
matmul work):
  - conv1 im2col is built on the HOST in per-chunk order (chunk m = samples
    4m..4m+3 via the block-diagonal group trick), DMAed in 16 fine-grained
    chunks so the first matmul starts at ~4us instead of 11us.
  - conv2 blocks are emitted interleaved into the conv1 chunk loop (lag 2),
    so conv2 matmuls run while conv1 pooling drains instead of after it.
  - pooling max stages use tensor_tensor(max) (2x DVE perf mode for packed
    bf16) instead of scalar_tensor_tensor (no perf mode), and PSUM tiles span
    2 banks so one Act evacuation covers 2 matmul tiles.
  - LIF layer-1 is solved analytically across the 3 timesteps straight from
    the cur1 PSUM (s1_t thresholds 2, 4/3, 8/7 on cur1), FC2 runs all 3
    timesteps in one matmul set (N=192), and FC2/FC3 weights are pre-scaled
    by 0.5 on the host so the LIF v-update is a single scalar_tensor_tensor.
"""

import numpy as np
import ml_dtypes

import concourse.bass as bass
import concourse.bacc as bacc
import concourse.tile as tile
import concourse.mybir as mybir

AF = mybir.ActivationFunctionType
ALU = mybir.AluOpType
BF16 = mybir.dt.bfloat16
F32 = mybir.dt.float32

NCORES = 8
B = 64  # samples per core


def _v(ap, p0, npart, dims, off=0):
    """View into an SBUF/PSUM tile AP: partition slice [p0, p0+npart) + custom free dims."""
    pitch = ap.ap[0][0]
    return bass.AP(
        tensor=ap.tensor,
        offset=ap.offset + p0 * pitch + off,
        ap=[[pitch, npart]] + [list(d) for d in dims],
    )


def _dv(ap, off, dims):
    """View into a DRAM tensor AP with custom dims."""
    return bass.AP(tensor=ap.tensor, offset=ap.offset + off, ap=[list(d) for d in dims])


def _emit(tc, io):
    nc = tc.nc
    from contextlib import ExitStack

    with ExitStack() as ctx:
        # ---------------- persistent buffers + weights ----------------
        # weights ride the Activation HWDGE queue: Pool stays free for buf96 descriptor
        # generation and SP for the im2col stream
        wp = ctx.enter_context(tc.tile_pool(name="wts", bufs=1))
        w1sb = wp.tile([109, 128], BF16)
        nc.scalar.dma_start(w1sb[:, :], io["w1l"][:, :])
        w2sb = wp.tile([97, 192], BF16)
        nc.scalar.dma_start(w2sb[:, :], io["w2l"][:, :])
        w3asb = wp.tile([128, 384], BF16)
        nc.scalar.dma_start(w3asb[:, :], io["w3a"][:, :])
        w3bsb = wp.tile([65, 384], BF16)
        nc.scalar.dma_start(w3bsb[:, :], io["w3b"][:, :])
        wf2sb = wp.tile([128, 512], BF16)
        nc.scalar.dma_start(wf2sb[:, :], io["wf2"][:, :])
        wf3sb = wp.tile([128, 5], BF16)
        nc.scalar.dma_start(wf3sb[:, :], io["wf3"][:, :])

        # preload the Relu activation table while the head DMAs run
        scr = wp.tile([1, 8], BF16)
        nc.scalar.activation(_v(scr, 0, 1, [[1, 8]]), _v(w1sb, 0, 1, [[1, 8]]), AF.Relu)

        mp = ctx.enter_context(tc.tile_pool(name="main", bufs=1))
        # conv1 pooled output, padded 26x26; partition 32g+c = sample 4m+g at col m*676
        xpad2 = mp.tile([128, 16 * 676 + 4], BF16)
        for dims, off in [
            ([[676, 16], [1, 26]], 0),        # top row
            ([[676, 16], [1, 26]], 650),      # bottom row
            ([[676, 16], [26, 26]], 0),       # left col
            ([[676, 16], [26, 26]], 25),      # right col
            ([[1, 4]], 16 * 676),             # tail pad (im2col dx over-read)
        ]:
            nc.gpsimd.memset(_v(xpad2, 0, 128, dims, off), 0.0)
        # conv2 pooled output, padded 14x14, partition 64h+c holds samples of parity h
        xpad3 = mp.tile([128, 32 * 198 + 4], BF16)
        # conv3 pooled output (features): [128c, sample*36 + hw]
        feat = mp.tile([128, B * 36], BF16)

        # LIF state
        lifp = ctx.enter_context(tc.tile_pool(name="lif", bufs=1))
        zeros = lifp.tile([128, 64], F32)
        v2 = lifp.tile([128, 64], F32)
        v3 = lifp.tile([5, 64], F32)
        acc = lifp.tile([5, 64], F32)
        s1_all = lifp.tile([128, 768], BF16)   # [t*256 + cur1-col]
        s2_all = lifp.tile([128, 192], BF16)   # [t*64 + sample-col]

        # conv3 im2col buffers (row 64 of B = bias row)
        c3b = ctx.enter_context(tc.tile_pool(name="c3buf", bufs=1))
        bufA = [c3b.tile([128, 32 * 198 + 4], BF16, name=f"bufA{h}") for h in range(2)]
        bufB = [c3b.tile([65, 32 * 198 + 4], BF16, name=f"bufB{h}") for h in range(2)]

        def late_inits():
            # not needed until conv2/the tail: emitted on the gpsimd queue after
            # the first buf96 pair DMAs so they don't delay the conv2 start
            for dims, off in [
                ([[198, 32], [1, 14]], 0),        # top row
                ([[198, 32], [1, 14]], 182),      # bottom row
                ([[198, 32], [14, 14]], 0),       # left col
                ([[198, 32], [14, 14]], 13),      # right col
                ([[1, 4]], 32 * 198),             # tail pad (im2col dx over-read)
                ([[198, 32], [1, 2]], 196),       # per-sample slack (pitch 198 vs 196)
            ]:
                nc.gpsimd.memset(_v(xpad3, 0, 128, dims, off), 0.0)
            for t in (zeros, v2, v3, acc):
                nc.gpsimd.memset(t[:, :], 0.0)
            for h in range(2):
                nc.gpsimd.dma_start(_v(bufB[h], 64, 1, [[1, 32 * 198 + 4]]), io["ones"][0:1, 0 : 32 * 198 + 4])

        # FC1 weights: loaded in 4 chunks spread across the conv1/conv2 window
        # (a single 13us DMA would block the serialized DMA engines)
        fcw = ctx.enter_context(tc.tile_pool(name="fcw", bufs=1))
        wf1sb = fcw.tile([128, 18432], BF16)

        # conv2 im2col quarters (96 rows = 32c x 3dx, row 96 = bias row), scoped
        b96 = ctx.enter_context(tc.tile_pool(name="b96", bufs=2))
        bqs = {}

        # ---------------- conv1 + conv2 + conv3 (interleaved, PE stays fed) ----------------
        with (
            tc.tile_pool(name="c2ps", bufs=2, space="PSUM") as c2p,
            tc.tile_pool(name="c2t", bufs=3) as c2t,
        ):
            def conv1_chunk(m, c1i, c1p, c1t):
                imct = c1i.tile([109, 2400], BF16, tag="imc", name="imc")
                nc.sync.dma_start(
                    _v(imct, 0, 109, [[1, 2400]]),
                    _dv(io["imc"], m * 2500, [[40000, 109], [1, 2400]]),
                )
                base = m * 676 + 27
                for tj in range(3):
                    ps = c1p.tile([128, 1024], F32, tag="ps1", name="ps1")
                    for u in range(2):
                        nc.tensor.matmul(
                            _v(ps, 0, 128, [[1, 384]], u * 512),
                            _v(w1sb, 0, 109, [[1, 128]]),
                            _v(imct, 0, 109, [[50, 8], [1, 48]], (tj * 2 + u) * 400),
                            start=True,
                            stop=True,
                        )
                    if tj < 2:
                        # Act evac: relu+copy both banks, x-deinterleaved (u,y,xh,phase)
                        stg = c1t.tile([128, 768], BF16, tag="stg", name="stg")
                        nc.scalar.activation(
                            _v(stg, 0, 128, [[192, 2], [24, 8], [1, 24], [384, 2]]),
                            _v(ps, 0, 128, [[512, 2], [48, 8], [2, 24], [1, 2]]),
                            AF.Relu,
                        )
                        # max stages as tensor_tensor (2x DVE mode on packed bf16)
                        xm = c1t.tile([128, 384], BF16, tag="xm", name="xm")
                        nc.vector.tensor_tensor(
                            _v(xm, 0, 128, [[1, 384]]),
                            _v(stg, 0, 128, [[1, 384]]),
                            _v(stg, 0, 128, [[1, 384]], 384),
                            ALU.max,
                        )
                        nc.vector.tensor_tensor(
                            _v(xpad2, 0, 128, [[26, 8], [1, 24]], base + tj * 8 * 26),
                            _v(xm, 0, 128, [[48, 8], [1, 24]]),
                            _v(xm, 0, 128, [[48, 8], [1, 24]], 24),
                            ALU.max,
                        )
                    else:
                        # DVE: direct 2x2 max-reduce from PSUM (relu deferred)
                        for u in range(2):
                            nc.vector.tensor_reduce(
                                _v(xpad2, 0, 128, [[26, 4], [1, 24]], base + (16 + 4 * u) * 26),
                                _v(ps, 0, 128, [[96, 4], [2, 24], [48, 2], [1, 2]], u * 512),
                                mybir.AxisListType.XY,
                                ALU.max,
                            )
                        rows = _v(xpad2, 0, 128, [[26, 8], [1, 24]], base + 16 * 26)
                        nc.vector.tensor_scalar(rows, rows, 0.0, None, ALU.max)
                # conv2 im2col: batched per chunk-PAIR (8 samples) on the gpsimd
                # SWDGE queue; quarter col layout is (g, chunk): sample
                # 16Q+4c+g at col (4g+c)*676
                Q = m // 4
                if m % 4 == 0:
                    bq = b96.tile([97, 16 * 676], BF16, tag="bq", name="bq")
                    bqs[Q] = bq
                    nc.gpsimd.dma_start(_v(bq, 96, 1, [[1, 16 * 676]]), io["ones"][0:1, 0 : 16 * 676])
                if m % 2 == 1:
                    bq = bqs[Q]
                    c0 = 2 * ((m // 2) % 2)
                    for g in range(4):
                        nc.gpsimd.dma_start(
                            _v(bq, 0, 96, [[1, 1352]], (4 * g + c0) * 676),
                            _v(xpad2, 32 * g, 32, [[1, 3], [1, 1352]], (m - 1) * 676),
                        )
                    # a 2.3KB-per-partition slice of the FC1 weights rides along
                    q8 = m // 2
                    nc.gpsimd.dma_start(
                        wf1sb[:, q8 * 2304 : (q8 + 1) * 2304],
                        _dv(io["wf1"], q8 * 2304, [[18432, 128], [1, 2304]]),
                    )

            def conv2_block(b):
                bq = bqs[b // 8]
                ps = c2p.tile([128, 1024], F32, tag="ps2", name="ps2")
                for yh in range(2):
                    for h in range(2):
                        s = 2 * b + h
                        loc = 4 * (s % 4) + (s // 4 - 4 * (b // 8))
                        for dy in range(3):
                            nc.tensor.matmul(
                                _v(ps, 64 * h, 64, [[1, 288]], yh * 512),
                                w2sb[0:97, dy * 64 : dy * 64 + 64],
                                _v(bq, 0, 97, [[26, 12], [1, 24]], loc * 676 + yh * 312 + dy * 26),
                                start=(dy == 0),
                                stop=(dy == 2),
                                tile_position=(0, 64 * h),
                            )
                # Act evac both banks (yh,y,xh,phase), then 2x tt max stages
                stg = c2t.tile([128, 576], BF16, tag="stg", name="stg")
                nc.scalar.activation(
                    _v(stg, 0, 128, [[144, 2], [12, 12], [1, 12], [288, 2]]),
                    _v(ps, 0, 128, [[512, 2], [24, 12], [2, 12], [1, 2]]),
                    AF.Relu,
                )
                xm = c2t.tile([128, 288], BF16, tag="xm", name="xm")
                nc.vector.tensor_tensor(
                    _v(xm, 0, 128, [[1, 288]]),
                    _v(stg, 0, 128, [[1, 288]]),
                    _v(stg, 0, 128, [[1, 288]], 288),
                    ALU.max,
                )
                nc.vector.tensor_tensor(
                    _v(xpad3, 0, 128, [[14, 12], [1, 12]], b * 198 + 15),
                    _v(xm, 0, 128, [[24, 12], [1, 12]]),
                    _v(xm, 0, 128, [[24, 12], [1, 12]], 12),
                    ALU.max,
                )
                # conv3 im2col chunk once its xpad3 sample range is complete
                if b == 15 or b == 31:
                    ck = b // 16
                    off = ck * 16 * 198
                    for h in range(2):
                        nc.gpsimd.dma_start(
                            _v(bufA[h], 0, 128, [[1, 16 * 198]], off),
                            _v(xpad3, 64 * h, 64, [[1, 2], [1, 16 * 198]], off),
                        )
                        nc.gpsimd.dma_start(
                            _v(bufB[h], 0, 64, [[1, 16 * 198]], off),
                            _v(xpad3, 64 * h, 64, [[1, 16 * 198]], off + 2),
                        )

            def conv3_unit(h, bp):
                # c3p/c3t are opened after the conv1 PSUM pool closes (bank budget)
                bj = bp % 4
                ps = c3p.tile([128, 288], F32, tag="ps3", name="ps3")
                for dy in range(3):
                    dims = [[198, 2], [14, 12], [1, 12]]
                    off = bp * 2 * 198 + dy * 14
                    nc.tensor.matmul(
                        ps[:, :], w3asb[0:128, dy * 128 : dy * 128 + 128],
                        _v(bufA[h], 0, 128, dims, off),
                        start=(dy == 0), stop=False,
                    )
                    nc.tensor.matmul(
                        ps[:, :], w3bsb[0:65, dy * 128 : dy * 128 + 128],
                        _v(bufB[h], 0, 65, dims, off),
                        start=False, stop=(dy == 2),
                    )
                # slot of (h, bp, i) is sample 4bp+h+2i -> feat col (4bp+h+2i)*36
                if bj < 3:
                    stg = c3t.tile([128, 288], BF16, tag="stg", name="stg")
                    nc.scalar.activation(
                        _v(stg, 0, 128, [[72, 2], [6, 12], [1, 6], [144, 2]]),
                        ps[:, :], AF.Relu,
                    )
                    xm = c3t.tile([128, 144], BF16, tag="xm", name="xm")
                    nc.vector.tensor_tensor(
                        _v(xm, 0, 128, [[1, 144]]),
                        _v(stg, 0, 128, [[1, 144]]),
                        _v(stg, 0, 128, [[1, 144]], 144),
                        ALU.max,
                    )
                    nc.vector.tensor_tensor(
                        _v(feat, 0, 128, [[72, 2], [6, 6], [1, 6]], (4 * bp + h) * 36),
                        _v(xm, 0, 128, [[72, 2], [12, 6], [1, 6]]),
                        _v(xm, 0, 128, [[72, 2], [12, 6], [1, 6]], 6),
                        ALU.max,
                    )
                else:
                    # DVE direct reduce per sample (relu deferred to feat pass)
                    for i in range(2):
                        nc.vector.tensor_reduce(
                            _v(feat, 0, 128, [[6, 6], [1, 6]], (4 * bp + h + 2 * i) * 36),
                            _v(ps, 0, 128, [[24, 6], [2, 6], [12, 2], [1, 2]], i * 144),
                            mybir.AxisListType.XY,
                            ALU.max,
                        )

            with (
                tc.tile_pool(name="c1imc", bufs=5) as c1i,
                tc.tile_pool(name="c1ps", bufs=2, space="PSUM") as c1p,
                tc.tile_pool(name="c1t", bufs=2) as c1t,
            ):
                for m in range(16):
                    conv1_chunk(m, c1i, c1p, c1t)
                    if m == 2:
                        late_inits()
                    if m >= 2:
                        conv2_block(2 * (m - 2))
                        conv2_block(2 * (m - 2) + 1)
            for b in range(28, 32):
                conv2_block(b)

        # ---------------- conv3 + FC1 (parity-pipelined) ----------------
        c3p = ctx.enter_context(tc.tile_pool(name="c3ps", bufs=3, space="PSUM"))
        c3t = ctx.enter_context(tc.tile_pool(name="c3t", bufs=4))
        cur1p = ctx.enter_context(tc.tile_pool(name="cur1p", bufs=1, space="PSUM"))
        cur1 = cur1p.tile([128, 256], F32)
        # ck0 units first (their im2col chunk landed at b=15); ck1's chunk
        # (emitted at b=31) transfers while these run
        for bp in range(8):
            for h in range(2):
                conv3_unit(h, bp)
        # s1_t straight from cur1: v=(v+c)/2, th=1, hard reset =>
        # s1_t1 = [c>=2]; s1_t2 = [c>=4/3]; s1_t3 = [c>=8/7] - [c>=4/3] + [c>=2]
        c43 = float(np.float32(4.0) / np.float32(3.0))
        c87 = float(np.float32(8.0) / np.float32(7.0))
        t87 = lifp.tile([128, 256], BF16)

        for h in range(2):
            for bp in range(8, 16):
                conv3_unit(h, bp)
            # in-place relu over the DVE-reduced feat slots of this parity
            # (bp = 3,7,11,15 -> slots 4bp+h and 4bp+2+h)
            rows = _v(feat, 0, 128, [[576, 4], [72, 2], [1, 36]], (12 + h) * 36)
            nc.vector.tensor_scalar(rows, rows, 0.0, None, ALU.max)
            # FC1 for parity h: out [unit, 32 samples] at cur1 col 64g+32h
            # (samples of parity h = feat cols h, h+2, ... -> stride 72)
            for g in range(4):
                for k in range(36):
                    nc.tensor.matmul(
                        cur1[:, 64 * g + 32 * h : 64 * g + 32 * h + 32],
                        wf1sb[:, k * 512 + g * 128 : k * 512 + g * 128 + 128],
                        _v(feat, 0, 128, [[72, 32]], k + 36 * h),
                        start=(k == 0),
                        stop=(k == 35),
                    )
            # LIF layer-1 thresholds for this parity's cur1 columns (overlaps
            # with the other parity's conv3/FC1)
            cslc = _v(cur1, 0, 128, [[64, 4], [1, 32]], 32 * h)
            s1t = lambda t, off=0: _v(s1_all, 0, 128, [[64, 4], [1, 32]], t * 256 + 32 * h + off)
            nc.vector.tensor_scalar(s1t(0), cslc, 2.0, None, ALU.is_ge)
            nc.vector.tensor_scalar(s1t(1), cslc, c43, None, ALU.is_ge)
            t87s = _v(t87, 0, 128, [[64, 4], [1, 32]], 32 * h)
            nc.vector.tensor_scalar(t87s, cslc, c87, None, ALU.is_ge)
            nc.vector.tensor_tensor(s1t(2), t87s, s1t(1), ALU.subtract)
            nc.vector.tensor_tensor(s1t(2), s1t(2), s1t(0), ALU.add)

        # ---------------- LIF + FC2/FC3 (layer-1 solved analytically) ----------------
        with (
            tc.tile_pool(name="cur2p", bufs=1, space="PSUM") as cur2p,
            tc.tile_pool(name="liftmp", bufs=2) as dtp,
        ):
            # FC2 for all 3 timesteps at once: N=192 (wf2 pre-scaled 0.5 on host)
            cur2 = cur2p.tile([128, 192], F32, tag="cur2")
            for g in range(4):
                nc.tensor.matmul(
                    cur2[:, :], wf2sb[:, g * 128 : g * 128 + 128],
                    _v(s1_all, 0, 128, [[256, 3], [1, 64]], 64 * g),
                    start=(g == 0), stop=(g == 3),
                )

            cur3 = cur2p.tile([5, 192], F32, tag="cur3")

            def lif2(t):
                # v2 <- v2*0.5 + cur2_half ; s2 = (v2 >= 1) ; v2 <- 0 where s2
                nc.vector.scalar_tensor_tensor(
                    v2[:, :], v2[:, :], 0.5, cur2[:, 64 * t : 64 * t + 64], ALU.mult, ALU.add)
                s2t = _v(s2_all, 0, 128, [[1, 64]], 64 * t)
                nc.vector.tensor_scalar(s2t, v2[:, :], 1.0, None, ALU.is_ge)
                nc.vector.copy_predicated(v2[:, :], s2t.bitcast(mybir.dt.uint16), zeros[:, :])
                nc.tensor.matmul(
                    cur3[0:5, 64 * t : 64 * t + 64], wf3sb[0:128, 0:5],
                    _v(s2_all, 0, 128, [[1, 64]], 64 * t),
                    start=True, stop=True,
                )

            def lif3(t):
                nc.vector.scalar_tensor_tensor(
                    v3[:, :], v3[:, :], 0.5, cur3[0:5, 64 * t : 64 * t + 64], ALU.mult, ALU.add)
                s3 = dtp.tile([5, 64], F32, tag="s3")
                nc.vector.tensor_scalar(s3[:, :], v3[:, :], 1.0, None, ALU.is_ge)
                nc.vector.copy_predicated(v3[:, :], s3[:, :].bitcast(mybir.dt.uint32), zeros[0:5, 0:64])
                nc.vector.tensor_tensor(acc[:, :], acc[:, :], s3[:, :], ALU.add)

            lif2(0)
            lif2(1)
            lif3(0)
            lif2(2)
            lif3(1)
            lif3(2)

            # acc/3 for acc in {0,1,2,3}: mult by fp32(1/3) matches true division except acc=3
            # (3*0.33333334 = 1.0000001) -> clamp with min(., 1.0) for exactness.
            nc.vector.tensor_scalar(acc[:, :], acc[:, :], float(np.float32(1.0) / np.float32(3.0)), 1.0, ALU.mult, ALU.min)
            # acc col (32h + j) holds sample 2j+h -> un-permute on the way out
            for h in range(2):
                nc.sync.dma_start(
                    _dv(io["out"], h, [[64, 5], [2, 32]]),
                    _v(acc, 0, 5, [[1, 32]], 32 * h),
                )


def _build():
    nc = bacc.Bacc("TRN2", target_bir_lowering=False, debug=False, enable_asserts=True)
    io = {}

    def inp(name, shape, dt):
        io[name] = nc.dram_tensor(name, shape, dt, kind="ExternalInput").ap()

    inp("imc", [109, 40000], BF16)
    inp("w1l", [109, 128], BF16)
    inp("w2l", [97, 192], BF16)
    inp("w3a", [128, 384], BF16)
    inp("w3b", [65, 384], BF16)
    inp("wf1", [128, 18432], BF16)
    inp("wf2", [128, 512], BF16)
    inp("wf3", [128, 5], BF16)
    inp("ones", [1, 10816], BF16)
    io["out"] = nc.dram_tensor("out", [5, 64], F32, kind="ExternalOutput").ap()

    with tile.TileContext(nc) as tc:
        _emit(tc, io)
    nc.compile()
    return nc


def _fake_quant(w):
    w = np.asarray(w, np.float32)
    scale = np.float32(np.max(np.abs(w)) / np.float32(127.0))
    wq = np.clip(np.round(w / scale), -127.0, 127.0).astype(np.float32) * scale
    return wq.astype(np.float32)


def _bf16(a):
    return np.asarray(a, np.float32).astype(ml_dtypes.bfloat16)


def _prep_weights(conv1_w, conv1_b, conv2_w, conv2_b, conv3_w, conv3_b, W1, W2, W3):
    c1 = np.asarray(conv1_w, np.float32)  # [32, 3, 3, 3]
    c2 = np.asarray(conv2_w, np.float32)  # [64, 32, 3, 3]
    c3 = np.asarray(conv3_w, np.float32)  # [128, 64, 3, 3]

    # conv1 block-diagonal: rows 27g..27g+26 = taps of group g -> cols 32g..32g+31;
    # row 108 = bias (tiled 4x over the 4 col groups).
    w1l = np.zeros((109, 128), np.float32)
    wk = c1.transpose(2, 3, 1, 0).reshape(27, 32)  # [(dy,dx,c), m]
    for q in range(4):
        w1l[27 * q : 27 * q + 27, 32 * q : 32 * q + 32] = wk
    w1l[108, :] = np.tile(np.asarray(conv1_b, np.float32), 4)

    w2l = np.zeros((97, 192), np.float32)
    w2l[0:96] = c2.transpose(1, 3, 2, 0).reshape(96, 192)  # [(c,dx), (dy,m)]
    w2l[96, 0:64] = np.asarray(conv2_b, np.float32)        # bias rides the dy=0 block

    w3x = c3.transpose(1, 3, 2, 0)  # [c, dx, dy, m]
    w3a = w3x[:, 0:2].reshape(128, 384)
    w3b = np.zeros((65, 384), np.float32)
    w3b[0:64] = w3x[:, 2].reshape(64, 384)
    w3b[64, 0:128] = np.asarray(conv3_b, np.float32)       # bias rides the dy=0 block

    W1q = _fake_quant(W1)  # [512, 4608]
    W2q = _fake_quant(W2)  # [128, 512]
    W3q = _fake_quant(W3)  # [5, 128]

    # [c, k*512 + u] = W1q[u, c*36 + k]  (FC1 weight-stationary: out [unit, sample])
    wf1 = W1q.reshape(512, 128, 36).transpose(1, 2, 0).reshape(128, 36 * 512)
    # FC2/FC3 pre-scaled by 0.5: LIF v-update becomes v*0.5 + cur_half in one op
    wf2 = 0.5 * W2q.T.reshape(4, 128, 128).transpose(1, 0, 2).reshape(128, 512)
    wf3 = 0.5 * W3q.T  # [128, 5]

    return {
        "w1l": _bf16(w1l),
        "w2l": _bf16(w2l),
        "w3a": _bf16(w3a),
        "w3b": _bf16(w3b),
        "wf1": _bf16(wf1),
        "wf2": _bf16(wf2),
        "wf3": _bf16(wf3),
        "ones": _bf16(np.ones((1, 10816), np.float32)),
    }


_NC = None
LAST_RESULTS = None


def kernel(x, conv1_w, conv1_b, conv2_w, conv2_b, conv3_w, conv3_b, W1, W2, W3, _trace=False):
    global _NC, LAST_RESULTS
    if _NC is None:
        _NC = _build()

    wmap = _prep_weights(conv1_w, conv1_b, conv2_w, conv2_b, conv3_w, conv3_b, W1, W2, W3)

    # host-side im2col for conv1, chunk-ordered: chunk m = samples 4m..4m+3,
    # imc[27g + dy*9 + dx*3 + c, m*2500 + p] = xpad[4m+g, c, p + dy*50 + dx]
    x = np.asarray(x, np.float32)
    xp = np.zeros((512, 3, 50, 50), np.float32)
    xp[:, :, 1:49, 1:49] = x
    xf = np.zeros((512, 3, 2604), np.float32)
    xf[:, :, :2500] = xp.reshape(512, 3, 2500)

    in_maps = []
    for i in range(NCORES):
        S = xf[B * i : B * (i + 1)].reshape(16, 4, 3, 2604)
        A = np.zeros((109, 40000), np.float32)
        for dy in range(3):
            for dx in range(3):
                blk = S[:, :, :, dy * 50 + dx : dy * 50 + dx + 2500]  # [m, g, c, p]
                for g in range(4):
                    r = 27 * g + dy * 9 + dx * 3
                    A[r : r + 3] = blk[:, g].transpose(1, 0, 2).reshape(3, 40000)
        A[108] = 1.0
        in_maps.append({"imc": _bf16(A), **wmap})

    from concourse.bass_utils import run_bass_kernel_spmd

    res = run_bass_kernel_spmd(_NC, in_maps, core_ids=list(range(NCORES)), trace=_trace)
    LAST_RESULTS = res
    out = np.concatenate([np.asarray(res.results[i]["out"]).T for i in range(NCORES)], axis=0)
    return np.ascontiguousarray(out.astype(np.float32))


# revision 25
# speedup vs baseline: 1.2343x; 1.0254x over previous
"""Trainium2 Bass kernel for ConvFCNet (3x conv+pool -> int8-fakequant FC + LIF SNN head).

Data-parallel over 8 NeuronCores: batch 512 -> 64 samples/core, weights replicated.

v1 rework (from 156us baseline): the PE queue is kept continuously fed so the
tensor engine stays at full p-state and is the binding resource (~89us of
matmul work):
  - conv1 im2col is built on the HOST in per-chunk order (chunk m = samples
    4m..4m+3 via the block-diagonal group trick), DMAed in 16 fine-grained
    chunks so the first matmul starts at ~4us instead of 11us.
  - conv2 blocks are emitted interleaved into the conv1 chunk loop (lag 2),
    so conv2 matmuls run while conv1 pooling drains instead of after it.
  - pooling max stages use tensor_tensor(max) (2x DVE perf mode for packed
    bf16) instead of scalar_tensor_tensor (no perf mode), and PSUM tiles span
    2 banks so one Act evacuation covers 2 matmul tiles.
  - LIF layer-1 is solved analytically across the 3 timesteps straight from
    the cur1 PSUM (s1_t thresholds 2, 4/3, 8/7 on cur1), FC2 runs all 3
    timesteps in one matmul set (N=192), and FC2/FC3 weights are pre-scaled
    by 0.5 on the host so the LIF v-update is a single scalar_tensor_tensor.
"""

import numpy as np
import ml_dtypes

import concourse.bass as bass
import concourse.bacc as bacc
import concourse.tile as tile
import concourse.mybir as mybir

AF = mybir.ActivationFunctionType
ALU = mybir.AluOpType
BF16 = mybir.dt.bfloat16
F32 = mybir.dt.float32

NCORES = 8
B = 64  # samples per core


def _v(ap, p0, npart, dims, off=0):
    """View into an SBUF/PSUM tile AP: partition slice [p0, p0+npart) + custom free dims."""
    pitch = ap.ap[0][0]
    return bass.AP(
        tensor=ap.tensor,
        offset=ap.offset + p0 * pitch + off,
        ap=[[pitch, npart]] + [list(d) for d in dims],
    )


def _dv(ap, off, dims):
    """View into a DRAM tensor AP with custom dims."""
    return bass.AP(tensor=ap.tensor, offset=ap.offset + off, ap=[list(d) for d in dims])


def _emit(tc, io):
    nc = tc.nc
    from contextlib import ExitStack

    with ExitStack() as ctx:
        # ---------------- persistent buffers + weights ----------------
        # weights ride the Activation HWDGE queue: Pool stays free for buf96 descriptor
        # generation and SP for the im2col stream
        wp = ctx.enter_context(tc.tile_pool(name="wts", bufs=1))
        w1sb = wp.tile([109, 128], BF16)
        nc.scalar.dma_start(w1sb[:, :], io["w1l"][:, :])
        w2sb = wp.tile([97, 192], BF16)
        nc.scalar.dma_start(w2sb[:, :], io["w2l"][:, :])
        w3asb = wp.tile([128, 384], BF16)
        w3bsb = wp.tile([65, 384], BF16)
        wf2sb = wp.tile([128, 512], BF16)
        wf3sb = wp.tile([128, 5], BF16)

        # preload the Relu activation table while the head DMAs run
        scr = wp.tile([1, 8], BF16)
        nc.scalar.activation(_v(scr, 0, 1, [[1, 8]]), _v(w1sb, 0, 1, [[1, 8]]), AF.Relu)

        mp = ctx.enter_context(tc.tile_pool(name="main", bufs=1))
        # conv1 pooled output, padded 26x26; partition 32g+c = sample 4m+g at col m*676
        xpad2 = mp.tile([128, 16 * 676 + 4], BF16)
        for dims, off in [
            ([[676, 16], [1, 26]], 0),        # top row
            ([[676, 16], [1, 26]], 650),      # bottom row
            ([[676, 16], [26, 26]], 0),       # left col
            ([[676, 16], [26, 26]], 25),      # right col
            ([[1, 4]], 16 * 676),             # tail pad (im2col dx over-read)
        ]:
            nc.gpsimd.memset(_v(xpad2, 0, 128, dims, off), 0.0)
        # conv2 pooled output, padded 14x14, partition 64h+c holds samples of parity h
        xpad3 = mp.tile([128, 32 * 198 + 4], BF16)
        # conv3 pooled output (features): [128c, sample*36 + hw]
        feat = mp.tile([128, B * 36], BF16)

        # LIF state
        lifp = ctx.enter_context(tc.tile_pool(name="lif", bufs=1))
        zeros = lifp.tile([128, 64], F32)
        v2 = lifp.tile([128, 64], F32)
        v3 = lifp.tile([5, 64], F32)
        acc = lifp.tile([5, 64], F32)
        s1_all = lifp.tile([128, 768], BF16)   # [t*256 + cur1-col]
        s2_all = lifp.tile([128, 192], BF16)   # [t*64 + sample-col]

        # conv3 im2col buffers (row 64 of B = bias row)
        c3b = ctx.enter_context(tc.tile_pool(name="c3buf", bufs=1))
        bufA = [c3b.tile([128, 32 * 198 + 4], BF16, name=f"bufA{h}") for h in range(2)]
        bufB = [c3b.tile([65, 32 * 198 + 4], BF16, name=f"bufB{h}") for h in range(2)]

        def late_inits():
            # not needed until conv2/the tail: emitted on the gpsimd queue after
            # the first buf96 pair DMAs so they don't delay the conv2 start
            for dims, off in [
                ([[198, 32], [1, 14]], 0),        # top row
                ([[198, 32], [1, 14]], 182),      # bottom row
                ([[198, 32], [14, 14]], 0),       # left col
                ([[198, 32], [14, 14]], 13),      # right col
                ([[1, 4]], 32 * 198),             # tail pad (im2col dx over-read)
                ([[198, 32], [1, 2]], 196),       # per-sample slack (pitch 198 vs 196)
            ]:
                nc.gpsimd.memset(_v(xpad3, 0, 128, dims, off), 0.0)
            for t in (zeros, v2, v3, acc):
                nc.gpsimd.memset(t[:, :], 0.0)
            nc.gpsimd.dma_start(w3asb[:, :], io["w3a"][:, :])
            nc.gpsimd.dma_start(w3bsb[:, :], io["w3b"][:, :])
            nc.gpsimd.dma_start(wf2sb[:, :], io["wf2"][:, :])
            nc.gpsimd.dma_start(wf3sb[:, :], io["wf3"][:, :])
            for h in range(2):
                nc.gpsimd.dma_start(_v(bufB[h], 64, 1, [[1, 32 * 198 + 4]]), io["ones"][0:1, 0 : 32 * 198 + 4])

        # FC1 weights: loaded in 4 chunks spread across the conv1/conv2 window
        # (a single 13us DMA would block the serialized DMA engines)
        fcw = ctx.enter_context(tc.tile_pool(name="fcw", bufs=1))
        wf1sb = fcw.tile([128, 18432], BF16)

        # conv2 im2col quarters (96 rows = 32c x 3dx, row 96 = bias row), scoped
        b96 = ctx.enter_context(tc.tile_pool(name="b96", bufs=2))
        bqs = {}

        # ---------------- conv1 + conv2 + conv3 (interleaved, PE stays fed) ----------------
        with (
            tc.tile_pool(name="c2ps", bufs=2, space="PSUM") as c2p,
            tc.tile_pool(name="c2t", bufs=3) as c2t,
        ):
            def conv1_chunk(m, c1i, c1p, c1t):
                imct = c1i.tile([109, 2400], BF16, tag="imc", name="imc")
                nc.sync.dma_start(
                    _v(imct, 0, 109, [[1, 2400]]),
                    _dv(io["imc"], m * 2500, [[40000, 109], [1, 2400]]),
                )
                base = m * 676 + 27
                for tj in range(3):
                    ps = c1p.tile([128, 1024], F32, tag="ps1", name="ps1")
                    for u in range(2):
                        nc.tensor.matmul(
                            _v(ps, 0, 128, [[1, 384]], u * 512),
                            _v(w1sb, 0, 109, [[1, 128]]),
                            _v(imct, 0, 109, [[50, 8], [1, 48]], (tj * 2 + u) * 400),
                            start=True,
                            stop=True,
                        )
                    if tj < 2:
                        # Act evac: relu+copy both banks, x-deinterleaved (u,y,xh,phase)
                        stg = c1t.tile([128, 768], BF16, tag="stg", name="stg")
                        nc.scalar.activation(
                            _v(stg, 0, 128, [[192, 2], [24, 8], [1, 24], [384, 2]]),
                            _v(ps, 0, 128, [[512, 2], [48, 8], [2, 24], [1, 2]]),
                            AF.Relu,
                        )
                        # max stages as tensor_tensor (2x DVE mode on packed bf16)
                        xm = c1t.tile([128, 384], BF16, tag="xm", name="xm")
                        nc.vector.tensor_tensor(
                            _v(xm, 0, 128, [[1, 384]]),
                            _v(stg, 0, 128, [[1, 384]]),
                            _v(stg, 0, 128, [[1, 384]], 384),
                            ALU.max,
                        )
                        nc.vector.tensor_tensor(
                            _v(xpad2, 0, 128, [[26, 8], [1, 24]], base + tj * 8 * 26),
                            _v(xm, 0, 128, [[48, 8], [1, 24]]),
                            _v(xm, 0, 128, [[48, 8], [1, 24]], 24),
                            ALU.max,
                        )
                    else:
                        # DVE: direct 2x2 max-reduce from PSUM (relu deferred)
                        for u in range(2):
                            nc.vector.tensor_reduce(
                                _v(xpad2, 0, 128, [[26, 4], [1, 24]], base + (16 + 4 * u) * 26),
                                _v(ps, 0, 128, [[96, 4], [2, 24], [48, 2], [1, 2]], u * 512),
                                mybir.AxisListType.XY,
                                ALU.max,
                            )
                        rows = _v(xpad2, 0, 128, [[26, 8], [1, 24]], base + 16 * 26)
                        nc.vector.tensor_scalar(rows, rows, 0.0, None, ALU.max)
                # conv2 im2col: batched per chunk-PAIR (8 samples) on the gpsimd
                # SWDGE queue; quarter col layout is (g, chunk): sample
                # 16Q+4c+g at col (4g+c)*676
                Q = m // 4
                if m % 4 == 0:
                    bq = b96.tile([97, 16 * 676], BF16, tag="bq", name="bq")
                    bqs[Q] = bq
                    nc.gpsimd.dma_start(_v(bq, 96, 1, [[1, 16 * 676]]), io["ones"][0:1, 0 : 16 * 676])
                if m % 2 == 1:
                    bq = bqs[Q]
                    c0 = 2 * ((m // 2) % 2)
                    for g in range(4):
                        nc.gpsimd.dma_start(
                            _v(bq, 0, 96, [[1, 1352]], (4 * g + c0) * 676),
                            _v(xpad2, 32 * g, 32, [[1, 3], [1, 1352]], (m - 1) * 676),
                        )


            def conv2_block(b):
                bq = bqs[b // 8]
                ps = c2p.tile([128, 1024], F32, tag="ps2", name="ps2")
                for yh in range(2):
                    for h in range(2):
                        s = 2 * b + h
                        loc = 4 * (s % 4) + (s // 4 - 4 * (b // 8))
                        for dy in range(3):
                            nc.tensor.matmul(
                                _v(ps, 64 * h, 64, [[1, 288]], yh * 512),
                                w2sb[0:97, dy * 64 : dy * 64 + 64],
                                _v(bq, 0, 97, [[26, 12], [1, 24]], loc * 676 + yh * 312 + dy * 26),
                                start=(dy == 0),
                                stop=(dy == 2),
                                tile_position=(0, 64 * h),
                            )
                # Act evac both banks (yh,y,xh,phase), then 2x tt max stages
                stg = c2t.tile([128, 576], BF16, tag="stg", name="stg")
                nc.scalar.activation(
                    _v(stg, 0, 128, [[144, 2], [12, 12], [1, 12], [288, 2]]),
                    _v(ps, 0, 128, [[512, 2], [24, 12], [2, 12], [1, 2]]),
                    AF.Relu,
                )
                xm = c2t.tile([128, 288], BF16, tag="xm", name="xm")
                nc.vector.tensor_tensor(
                    _v(xm, 0, 128, [[1, 288]]),
                    _v(stg, 0, 128, [[1, 288]]),
                    _v(stg, 0, 128, [[1, 288]], 288),
                    ALU.max,
                )
                nc.vector.tensor_tensor(
                    _v(xpad3, 0, 128, [[14, 12], [1, 12]], b * 198 + 15),
                    _v(xm, 0, 128, [[24, 12], [1, 12]]),
                    _v(xm, 0, 128, [[24, 12], [1, 12]], 12),
                    ALU.max,
                )
                # conv3 im2col chunk once its xpad3 sample range is complete
                if b == 15 or b == 31:
                    ck = b // 16
                    off = ck * 16 * 198
                    for h in range(2):
                        nc.gpsimd.dma_start(
                            _v(bufA[h], 0, 128, [[1, 16 * 198]], off),
                            _v(xpad3, 64 * h, 64, [[1, 2], [1, 16 * 198]], off),
                        )
                        nc.gpsimd.dma_start(
                            _v(bufB[h], 0, 64, [[1, 16 * 198]], off),
                            _v(xpad3, 64 * h, 64, [[1, 16 * 198]], off + 2),
                        )

            def conv3_unit(h, bp):
                # c3p/c3t are opened after the conv1 PSUM pool closes (bank budget)
                bj = bp % 4
                ps = c3p.tile([128, 288], F32, tag="ps3", name="ps3")
                for dy in range(3):
                    dims = [[198, 2], [14, 12], [1, 12]]
                    off = bp * 2 * 198 + dy * 14
                    nc.tensor.matmul(
                        ps[:, :], w3asb[0:128, dy * 128 : dy * 128 + 128],
                        _v(bufA[h], 0, 128, dims, off),
                        start=(dy == 0), stop=False,
                    )
                    nc.tensor.matmul(
                        ps[:, :], w3bsb[0:65, dy * 128 : dy * 128 + 128],
                        _v(bufB[h], 0, 65, dims, off),
                        start=False, stop=(dy == 2),
                    )
                # slot of (h, bp, i) is sample 4bp+h+2i -> feat col (4bp+h+2i)*36
                if bj < 3:
                    stg = c3t.tile([128, 288], BF16, tag="stg", name="stg")
                    nc.scalar.activation(
                        _v(stg, 0, 128, [[72, 2], [6, 12], [1, 6], [144, 2]]),
                        ps[:, :], AF.Relu,
                    )
                    xm = c3t.tile([128, 144], BF16, tag="xm", name="xm")
                    nc.vector.tensor_tensor(
                        _v(xm, 0, 128, [[1, 144]]),
                        _v(stg, 0, 128, [[1, 144]]),
                        _v(stg, 0, 128, [[1, 144]], 144),
                        ALU.max,
                    )
                    nc.vector.tensor_tensor(
                        _v(feat, 0, 128, [[72, 2], [6, 6], [1, 6]], (4 * bp + h) * 36),
                        _v(xm, 0, 128, [[72, 2], [12, 6], [1, 6]]),
                        _v(xm, 0, 128, [[72, 2], [12, 6], [1, 6]], 6),
                        ALU.max,
                    )
                else:
                    # DVE direct reduce per sample (relu deferred to feat pass)
                    for i in range(2):
                        nc.vector.tensor_reduce(
                            _v(feat, 0, 128, [[6, 6], [1, 6]], (4 * bp + h + 2 * i) * 36),
                            _v(ps, 0, 128, [[24, 6], [2, 6], [12, 2], [1, 2]], i * 144),
                            mybir.AxisListType.XY,
                            ALU.max,
                        )

            with (
                tc.tile_pool(name="c1imc", bufs=5) as c1i,
                tc.tile_pool(name="c1ps", bufs=2, space="PSUM") as c1p,
                tc.tile_pool(name="c1t", bufs=2) as c1t,
            ):
                for m in range(16):
                    conv1_chunk(m, c1i, c1p, c1t)
                    if m == 2:
                        late_inits()
                    if m >= 2:
                        conv2_block(2 * (m - 2))
                        conv2_block(2 * (m - 2) + 1)
            for b in range(28, 32):
                conv2_block(b)

        # ---------------- conv3 + FC1 (parity-pipelined) ----------------
        c3p = ctx.enter_context(tc.tile_pool(name="c3ps", bufs=4, space="PSUM"))
        c3t = ctx.enter_context(tc.tile_pool(name="c3t", bufs=4))
        cur1p = ctx.enter_context(tc.tile_pool(name="cur1p", bufs=1, space="PSUM"))
        cur1 = cur1p.tile([128, 256], F32)
        # ck0 units first (their im2col chunk landed at b=15); ck1's chunk
        # (emitted at b=31) and the FC1 weights transfer while these run
        for bp in range(8):
            for h in range(2):
                conv3_unit(h, bp)
            nc.gpsimd.dma_start(
                wf1sb[:, bp * 2304 : (bp + 1) * 2304],
                _dv(io["wf1"], bp * 2304, [[18432, 128], [1, 2304]]),
            )
        # s1_t straight from cur1: v=(v+c)/2, th=1, hard reset =>
        # s1_t1 = [c>=2]; s1_t2 = [c>=4/3]; s1_t3 = [c>=8/7] - [c>=4/3] + [c>=2]
        c43 = float(np.float32(4.0) / np.float32(3.0))
        c87 = float(np.float32(8.0) / np.float32(7.0))
        t87 = lifp.tile([128, 256], BF16)

        def rows_relu(h):
            # in-place relu over the DVE-reduced feat slots of this parity
            # (bp = 3,7,11,15 -> slots 4bp+h and 4bp+2+h)
            rows = _v(feat, 0, 128, [[576, 4], [72, 2], [1, 36]], (12 + h) * 36)
            nc.vector.tensor_scalar(rows, rows, 0.0, None, ALU.max)

        def fc1(h):
            # FC1 for parity h: out [unit, 32 samples] at cur1 col 64g+32h
            # (samples of parity h = feat cols h, h+2, ... -> stride 72)
            for g in range(4):
                for k in range(36):
                    nc.tensor.matmul(
                        cur1[:, 64 * g + 32 * h : 64 * g + 32 * h + 32],
                        wf1sb[:, k * 512 + g * 128 : k * 512 + g * 128 + 128],
                        _v(feat, 0, 128, [[72, 32]], k + 36 * h),
                        start=(k == 0),
                        stop=(k == 35),
                    )

        def thresholds(h):
            # LIF layer-1 thresholds for this parity's cur1 columns
            cslc = _v(cur1, 0, 128, [[64, 4], [1, 32]], 32 * h)
            s1t = lambda t: _v(s1_all, 0, 128, [[64, 4], [1, 32]], t * 256 + 32 * h)
            nc.vector.tensor_scalar(s1t(0), cslc, 2.0, None, ALU.is_ge)
            nc.vector.tensor_scalar(s1t(1), cslc, c43, None, ALU.is_ge)
            t87s = _v(t87, 0, 128, [[64, 4], [1, 32]], 32 * h)
            nc.vector.tensor_scalar(t87s, cslc, c87, None, ALU.is_ge)
            nc.vector.tensor_tensor(s1t(2), t87s, s1t(1), ALU.subtract)
            nc.vector.tensor_tensor(s1t(2), s1t(2), s1t(0), ALU.add)

        # parity-0 ck1 units, then FC1(0) hidden behind the first parity-1 units
        for bp in range(8, 16):
            conv3_unit(0, bp)
        rows_relu(0)
        for bp in range(8, 11):
            conv3_unit(1, bp)
        fc1(0)
        thresholds(0)
        for bp in range(11, 16):
            conv3_unit(1, bp)
        rows_relu(1)
        fc1(1)
        thresholds(1)

        # ---------------- LIF + FC2/FC3 (layer-1 solved analytically) ----------------
        with (
            tc.tile_pool(name="cur2p", bufs=1, space="PSUM") as cur2p,
            tc.tile_pool(name="liftmp", bufs=2) as dtp,
        ):
            # FC2 for all 3 timesteps at once: N=192 (wf2 pre-scaled 0.5 on host)
            cur2 = cur2p.tile([128, 192], F32, tag="cur2")
            for g in range(4):
                nc.tensor.matmul(
                    cur2[:, :], wf2sb[:, g * 128 : g * 128 + 128],
                    _v(s1_all, 0, 128, [[256, 3], [1, 64]], 64 * g),
                    start=(g == 0), stop=(g == 3),
                )

            cur3 = cur2p.tile([5, 192], F32, tag="cur3")

            def lif2(t):
                # v2 <- v2*0.5 + cur2_half ; s2 = (v2 >= 1) ; v2 <- 0 where s2
                nc.vector.scalar_tensor_tensor(
                    v2[:, :], v2[:, :], 0.5, cur2[:, 64 * t : 64 * t + 64], ALU.mult, ALU.add)
                s2t = _v(s2_all, 0, 128, [[1, 64]], 64 * t)
                nc.vector.tensor_scalar(s2t, v2[:, :], 1.0, None, ALU.is_ge)
                nc.vector.copy_predicated(v2[:, :], s2t.bitcast(mybir.dt.uint16), zeros[:, :])
                nc.tensor.matmul(
                    cur3[0:5, 64 * t : 64 * t + 64], wf3sb[0:128, 0:5],
                    _v(s2_all, 0, 128, [[1, 64]], 64 * t),
                    start=True, stop=True,
                )

            def lif3(t):
                nc.vector.scalar_tensor_tensor(
                    v3[:, :], v3[:, :], 0.5, cur3[0:5, 64 * t : 64 * t + 64], ALU.mult, ALU.add)
                s3 = dtp.tile([5, 64], F32, tag="s3")
                nc.vector.tensor_scalar(s3[:, :], v3[:, :], 1.0, None, ALU.is_ge)
                nc.vector.copy_predicated(v3[:, :], s3[:, :].bitcast(mybir.dt.uint32), zeros[0:5, 0:64])
                nc.vector.tensor_tensor(acc[:, :], acc[:, :], s3[:, :], ALU.add)

            lif2(0)
            lif2(1)
            lif3(0)
            lif2(2)
            lif3(1)
            lif3(2)

            # acc/3 for acc in {0,1,2,3}: mult by fp32(1/3) matches true division except acc=3
            # (3*0.33333334 = 1.0000001) -> clamp with min(., 1.0) for exactness.
            nc.vector.tensor_scalar(acc[:, :], acc[:, :], float(np.float32(1.0) / np.float32(3.0)), 1.0, ALU.mult, ALU.min)
            # acc col (32h + j) holds sample 2j+h -> un-permute on the way out
            for h in range(2):
                nc.sync.dma_start(
                    _dv(io["out"], h, [[64, 5], [2, 32]]),
                    _v(acc, 0, 5, [[1, 32]], 32 * h),
                )


def _build():
    nc = bacc.Bacc("TRN2", target_bir_lowering=False, debug=False, enable_asserts=True)
    io = {}

    def inp(name, shape, dt):
        io[name] = nc.dram_tensor(name, shape, dt, kind="ExternalInput").ap()

    inp("imc", [109, 40000], BF16)
    inp("w1l", [109, 128], BF16)
    inp("w2l", [97, 192], BF16)
    inp("w3a", [128, 384], BF16)
    inp("w3b", [65, 384], BF16)
    inp("wf1", [128, 18432], BF16)
    inp("wf2", [128, 512], BF16)
    inp("wf3", [128, 5], BF16)
    inp("ones", [1, 10816], BF16)
    io["out"] = nc.dram_tensor("out", [5, 64], F32, kind="ExternalOutput").ap()

    with tile.TileContext(nc) as tc:
        _emit(tc, io)
    nc.compile()
    return nc


def _fake_quant(w):
    w = np.asarray(w, np.float32)
    scale = np.float32(np.max(np.abs(w)) / np.float32(127.0))
    wq = np.clip(np.round(w / scale), -127.0, 127.0).astype(np.float32) * scale
    return wq.astype(np.float32)


def _bf16(a):
    return np.asarray(a, np.float32).astype(ml_dtypes.bfloat16)


def _prep_weights(conv1_w, conv1_b, conv2_w, conv2_b, conv3_w, conv3_b, W1, W2, W3):
    c1 = np.asarray(conv1_w, np.float32)  # [32, 3, 3, 3]
    c2 = np.asarray(conv2_w, np.float32)  # [64, 32, 3, 3]
    c3 = np.asarray(conv3_w, np.float32)  # [128, 64, 3, 3]

    # conv1 block-diagonal: rows 27g..27g+26 = taps of group g -> cols 32g..32g+31;
    # row 108 = bias (tiled 4x over the 4 col groups).
    w1l = np.zeros((109, 128), np.float32)
    wk = c1.transpose(2, 3, 1, 0).reshape(27, 32)  # [(dy,dx,c), m]
    for q in range(4):
        w1l[27 * q : 27 * q + 27, 32 * q : 32 * q + 32] = wk
    w1l[108, :] = np.tile(np.asarray(conv1_b, np.float32), 4)

    w2l = np.zeros((97, 192), np.float32)
    w2l[0:96] = c2.transpose(1, 3, 2, 0).reshape(96, 192)  # [(c,dx), (dy,m)]
    w2l[96, 0:64] = np.asarray(conv2_b, np.float32)        # bias rides the dy=0 block

    w3x = c3.transpose(1, 3, 2, 0)  # [c, dx, dy, m]
    w3a = w3x[:, 0:2].reshape(128, 384)
    w3b = np.zeros((65, 384), np.float32)
    w3b[0:64] = w3x[:, 2].reshape(64, 384)
    w3b[64, 0:128] = np.asarray(conv3_b, np.float32)       # bias rides the dy=0 block

    W1q = _fake_quant(W1)  # [512, 4608]
    W2q = _fake_quant(W2)  # [128, 512]
    W3q = _fake_quant(W3)  # [5, 128]

    # [c, k*512 + u] = W1q[u, c*36 + k]  (FC1 weight-stationary: out [unit, sample])
    wf1 = W1q.reshape(512, 128, 36).transpose(1, 2, 0).reshape(128, 36 * 512)
    # FC2/FC3 pre-scaled by 0.5: LIF v-update becomes v*0.5 + cur_half in one op
    wf2 = 0.5 * W2q.T.reshape(4, 128, 128).transpose(1, 0, 2).reshape(128, 512)
    wf3 = 0.5 * W3q.T  # [128, 5]

    return {
        "w1l": _bf16(w1l),
        "w2l": _bf16(w2l),
        "w3a": _bf16(w3a),
        "w3b": _bf16(w3b),
        "wf1": _bf16(wf1),
        "wf2": _bf16(wf2),
        "wf3": _bf16(wf3),
        "ones": _bf16(np.ones((1, 10816), np.float32)),
    }


_NC = None
LAST_RESULTS = None


def kernel(x, conv1_w, conv1_b, conv2_w, conv2_b, conv3_w, conv3_b, W1, W2, W3, _trace=False):
    global _NC, LAST_RESULTS
    if _NC is None:
        _NC = _build()

    wmap = _prep_weights(conv1_w, conv1_b, conv2_w, conv2_b, conv3_w, conv3_b, W1, W2, W3)

    # host-side im2col for conv1, chunk-ordered: chunk m = samples 4m..4m+3,
    # imc[27g + dy*9 + dx*3 + c, m*2500 + p] = xpad[4m+g, c, p + dy*50 + dx]
    x = np.asarray(x, np.float32)
    xp = np.zeros((512, 3, 50, 50), np.float32)
    xp[:, :, 1:49, 1:49] = x
    xf = np.zeros((512, 3, 2604), np.float32)
    xf[:, :, :2500] = xp.reshape(512, 3, 2500)

    in_maps = []
    for i in range(NCORES):
        S = xf[B * i : B * (i + 1)].reshape(16, 4, 3, 2604)
        A = np.zeros((109, 40000), np.float32)
        for dy in range(3):
            for dx in range(3):
                blk = S[:, :, :, dy * 50 + dx : dy * 50 + dx + 2500]  # [m, g, c, p]
                for g in range(4):
                    r = 27 * g + dy * 9 + dx * 3
                    A[r : r + 3] = blk[:, g].transpose(1, 0, 2).reshape(3, 40000)
        A[108] = 1.0
        in_maps.append({"imc": _bf16(A), **wmap})

    from concourse.bass_utils import run_bass_kernel_spmd

    res = run_bass_kernel_spmd(_NC, in_maps, core_ids=list(range(NCORES)), trace=_trace)
    LAST_RESULTS = res
    out = np.concatenate([np.asarray(res.results[i]["out"]).T for i in range(NCORES)], axis=0)
    return np.ascontiguousarray(out.astype(np.float32))
